# revision 1
# baseline (speedup 1.0000x reference)
"""TRN2 Bass kernel for nn_AttnPlainNet (gnn_message_passing).

Math (C=1 collapses everything):
  l2norm over C=1  -> u = sign(x), sgn_nb = sign(neighbor)
  att weights      -> watt[b,n] = softmax_n(s_x[b]*s_y[b,n])
  v[b,f] = sum_n watt*sgn_nb ; w = u*v
  fadj[a,e] = u_a u_e S(w_a+w_e) / (d_e + eps),  S(t)=sign(t)sqrt|t|,
  d_e = sum_a sqrt|w_a+w_e|   (A = S-matrix is symmetric)
  layer1: z1[k] = u_k t_k/(d_k+eps), t_k = sum_f S(w_f+w_k)
  BN1 is affine in z1 (stats from global z1 mean/var -> 2-float all-reduce)
  p~ = softsign(alpha*z1+beta)*u ; layer2: z2[k,c] = u_k/(d_k+eps) *
        sum_f As[f,k] p~[f,c]  (PE matmul over cached As)
  BN2 stats from z2 first/second moments (16x17 all-reduce)
  q = softsign(W2' z2 + delta) ; out = q @ WcT + bc
Sharding: pure data-parallel, 32 batches per core, 8 cores.
"""
from contextlib import ExitStack

import numpy as np

import concourse.bass as bass
import concourse.mybir as mybir
import concourse.tile as tile
from concourse import bacc
from concourse.bass_utils import run_bass_kernel_spmd
from concourse.masks import make_identity

# Steer the act-table-set chooser away from the partial ln-only / exp-only
# sets so Ln+Exp sequences stay resident in natural_log_exp_and_others
# (positional set ids must be preserved, so entries are emptied, not removed).
_orig_get_tables = bacc.get_activation_tables


def _patched_get_tables(arch):
    tabs = dict(_orig_get_tables(arch))
    for name in ("natural_log", "exp_and_others", "exp_and_friends"):
        if name in tabs:
            tabs[name] = set()
    return tabs


bacc.get_activation_tables = _patched_get_tables

AF = mybir.ActivationFunctionType
ALU = mybir.AluOpType
F32 = mybir.dt.float32
F16 = mybir.dt.float16
U16 = mybir.dt.uint16

B, N, F, H, NCLS = 256, 32, 512, 16, 64
NCORES = 8
BL = B // NCORES          # 32 local batches
FC = 4                    # f/k chunks of 128
P = 128
EPS_ROW = 1e-7
EPS_BN = 1e-5
NK = float(B * F)         # BN normalizer (global)

_CACHE = {}


def _bc_ap(handle_ap, ap):
    """AP with explicit [stride, count] dims over a tensor handle's AP."""
    return bass.AP(tensor=handle_ap.tensor, offset=handle_ap.offset, ap=ap)


def build_program(no_cc=False):
    nc = bacc.Bacc("TRN2", num_devices=NCORES)

    # ---- I/O -------------------------------------------------------------
    x_l = nc.dram_tensor("x_l", [BL, F], F32, kind="ExternalInput")
    nb_l = nc.dram_tensor("nb_l", [BL * N, F], F32, kind="ExternalInput")
    att1 = nc.dram_tensor("att1", [1, F], F32, kind="ExternalInput")
    att2 = nc.dram_tensor("att2", [1, F], F32, kind="ExternalInput")
    w1c = nc.dram_tensor("w1c", [H, 1], F32, kind="ExternalInput")
    b1 = nc.dram_tensor("b1", [H, 1], F32, kind="ExternalInput")
    g1 = nc.dram_tensor("g1", [H, 1], F32, kind="ExternalInput")
    be1 = nc.dram_tensor("be1", [H, 1], F32, kind="ExternalInput")
    w2 = nc.dram_tensor("w2", [H, H], F32, kind="ExternalInput")
    w2t = nc.dram_tensor("w2t", [H, H], F32, kind="ExternalInput")
    b2 = nc.dram_tensor("b2", [H, 1], F32, kind="ExternalInput")
    g2 = nc.dram_tensor("g2", [H, 1], F32, kind="ExternalInput")
    be2 = nc.dram_tensor("be2", [H, 1], F32, kind="ExternalInput")
    wct = nc.dram_tensor("wct", [H * F, NCLS], F16, kind="ExternalInput")
    bc = nc.dram_tensor("bc", [1, NCLS], F32, kind="ExternalInput")
    out_l = nc.dram_tensor("out_l", [BL, NCLS], F32, kind="ExternalOutput")

    with tile.TileContext(nc) as tc, ExitStack() as ctx:
        sg = ctx.enter_context(tc.tile_pool(name="singles", bufs=1))
        wk = ctx.enter_context(tc.tile_pool(name="work", bufs=2))
        t2 = ctx.enter_context(tc.tile_pool(name="t2", bufs=2))
        bigp = ctx.enter_context(tc.tile_pool(name="big2", bufs=1))
        wbp = ctx.enter_context(tc.tile_pool(name="wbp", bufs=3))
        st1ctx = ExitStack()
        s1 = st1ctx.enter_context(tc.tile_pool(name="stage1", bufs=1))
        dr = ctx.enter_context(tc.tile_pool(name="dram", bufs=1, space="DRAM"))
        ps = ctx.enter_context(tc.tile_pool(name="psmall", bufs=2, space="PSUM"))
        pgt = ctx.enter_context(tc.tile_pool(name="pgt", bufs=2, space="PSUM"))
        pm2 = ctx.enter_context(tc.tile_pool(name="pm2", bufs=1, space="PSUM"))
        pq = ctx.enter_context(tc.tile_pool(name="pq", bufs=1, space="PSUM"))

        V, S, G = nc.vector, nc.scalar, nc.gpsimd
        TE = nc.tensor

        # ---- constants ---------------------------------------------------
        i32 = sg.tile([32, 32], F32)
        make_identity(nc, i32[:])
        i32h = sg.tile([32, 32], F16)
        make_identity(nc, i32h[:])
        i16 = sg.tile([16, 16], F32)
        make_identity(nc, i16[:])
        i16h = sg.tile([16, 16], F16)
        make_identity(nc, i16h[:])
        i128h = sg.tile([P, P], F16)
        make_identity(nc, i128h[:])
        epsb = sg.tile([H, 1], F32)
        V.memset(epsb[:], EPS_BN)
        ones128 = sg.tile([P, 1], F32)
        V.memset(ones128[:], 1.0)
        ones128h = sg.tile([P, 1], F16)
        V.memset(ones128h[:], 1.0)
        onesrow = sg.tile([1, P], F32)
        V.memset(onesrow[:], 1.0)
        blkones = sg.tile([P, 4], F32)
        V.memset(blkones[:], 0.0)
        for a in range(4):
            V.memset(blkones[32 * a:32 * a + 32, a:a + 1], 1.0)

        # broadcast att vectors
        att1_b = s1.tile([32, F], F32)
        nc.sync.dma_start(att1_b[:], _bc_ap(att1[:], [[0, 32], [1, F]]))
        att2_b = s1.tile([P, F], F32)
        nc.sync.dma_start(att2_b[:], _bc_ap(att2[:], [[0, P], [1, F]]))

        # WcT tiles [128, 64jc, 64n] fp16
        wct_sb = sg.tile([P, 64, NCLS], F16)
        nc.sync.dma_start(wct_sb[:], wct[:].rearrange("(jc p) n -> p jc n", p=P))
        bc_rep = sg.tile([8, NCLS], F32)
        nc.sync.dma_start(bc_rep[:], _bc_ap(bc[:], [[0, 8], [1, NCLS]]))

        # per-channel weights [16,1]
        w1s = sg.tile([H, 1], F32)
        nc.sync.dma_start(w1s[:], w1c[:])
        b1s = sg.tile([H, 1], F32)
        nc.sync.dma_start(b1s[:], b1[:])
        g1s = sg.tile([H, 1], F32)
        nc.sync.dma_start(g1s[:], g1[:])
        be1s = sg.tile([H, 1], F32)
        nc.sync.dma_start(be1s[:], be1[:])
        b2s = sg.tile([H, 1], F32)
        nc.sync.dma_start(b2s[:], b2[:])
        g2s = sg.tile([H, 1], F32)
        nc.sync.dma_start(g2s[:], g2[:])
        be2s = sg.tile([H, 1], F32)
        nc.sync.dma_start(be2s[:], be2[:])
        w2s = sg.tile([H, H], F32)
        nc.sync.dma_start(w2s[:], w2[:])
        w2ts = sg.tile([H, H], F32)
        nc.sync.dma_start(w2ts[:], w2t[:])

        # ---- stage 0: x -> u, s_x ---------------------------------------
        xsb = wk.tile([P, F], F32, tag="nbt")
        nc.sync.dma_start(xsb[0:BL, :], x_l[:])
        u32 = sg.tile([BL, F], F32)
        S.activation(u32[:], xsb[0:BL, :], AF.Sign)
        sx_col = sg.tile([BL, 1], F32)
        V.scalar_tensor_tensor(xsb[0:BL, :], u32[:], 0.0, att1_b[:],
                               ALU.bypass, ALU.mult, accum_out=sx_col[:])

        # ---- stage 1 (pipelined per tile): sgn, s_y, softmax, v, w ----
        i4 = sg.tile([4, 4], F32)
        make_identity(nc, i4[:])
        sx_d = dr.tile([BL], F32)
        nc.sync.dma_start(sx_d[:], sx_col[:].rearrange("b one -> (b one)"))
        sx_rep = sg.tile([P, 8], F32)
        for a in range(4):
            nc.sync.dma_start(sx_rep[32 * a:32 * a + 32, :],
                        bass.AP(tensor=sx_d[:].tensor,
                                offset=sx_d[:].offset + a,
                                ap=[[0, 32], [4, 8]]))
        w16_ds = [dr.tile([4, F], F16, tag=f"w16d{j}", name=f"w16d{j}") for j in range(8)]
        wT_js = [sg.tile([P, 16], F32, tag=f"wtj{j}", name=f"wtj{j}") for j in range(8)]
        for j in range(8):
            nbt = wk.tile([P, F], F32, tag="nbt")
            nc.sync.dma_start(nbt[:], nb_l[:].rearrange("(j p) f -> j p f", p=P)[j])
            sgn = wk.tile([P, F], F32, tag="sgn")
            S.activation(sgn[:], nbt[:], AF.Sign)
            sy = wk.tile([P, 1], F32, tag="sy")
            V.scalar_tensor_tensor(nbt[:], sgn[:], 0.0, att2_b[:],
                                   ALU.bypass, ALU.mult, accum_out=sy[:])
            lcol = wk.tile([P, 1], F32, tag="lcol")
            V.tensor_tensor(lcol[:], sy[:], sx_rep[:, j:j + 1], ALU.mult)
            ecol = wk.tile([P, 1], F32, tag="ecol")
            S.activation(ecol[:], lcol[:], AF.Exp)
            p_dn = ps.tile([4, 1], F32, tag="sm")
            TE.matmul(p_dn[:], blkones[:], ecol[:], start=True, stop=True)
            rdn = wk.tile([4, 1], F32, tag="rdn")
            V.reciprocal(rdn[:], p_dn[:])
            wd4 = wk.tile([P, 4], F32, tag="wd")
            V.tensor_tensor(wd4[:], ecol[:].to_broadcast([P, 4]),
                            blkones[:], ALU.mult)
            p_vj = ps.tile([4, F], F32, tag="sm")
            TE.matmul(p_vj[:], wd4[:], sgn[:], start=True, stop=True)
            u_j = wk.tile([4, F], F32, tag="uj")
            nc.sync.dma_start(u_j[:], u32[4 * j:4 * j + 4, :])
            w_j = wk.tile([4, F], F32, tag="wj")
            V.tensor_scalar(w_j[:], p_vj[:], rdn[:], None, ALU.mult)
            V.tensor_tensor(w_j[:], w_j[:], u_j[:], ALU.mult)
            w16_j = wk.tile([4, F], F16, tag="w16j")
            V.tensor_copy(w16_j[:], w_j[:])
            nc.sync.dma_start(w16_ds[j][:], w16_j[:])
            p_wt = ps.tile([P, 4, 4], F32, tag="sm")
            for c in range(FC):
                TE.transpose(p_wt[:, c, :], w_j[:, P * c:P * c + P], i4[:])
            V.tensor_copy(wT_js[j][:], p_wt[:])

        # u transpose (for BN1/ptil later)
        p_tu = ps.tile([P, P], F32, tag="sm")
        for c in range(FC):
            TE.transpose(p_tu[:, 32 * c:32 * c + 32],
                         u32[:, P * c:P * c + P], i32[:])
        uT = sg.tile([P, P], F32)
        V.tensor_copy(uT[:], p_tu[:])
        st1ctx.close()

        # ---- stage 2: main pass-1 loop (A matrix, d, t, As cache) --------
        as_cache = sg.tile([P, FC, BL, F], F16)
        onehot = sg.tile([P, 63], F16)
        V.memset(onehot[:], 0.0)
        V.memset(onehot[:, 31:32], 1.0)
        p_t32 = pm2.tile([BL, F], F32, tag="pm2")
        p_d32 = pm2.tile([BL, F], F32, tag="pm1")
        for b in range(BL):
            w_bc = wbp.tile([P, F], F16, tag="wbc")
            wd_ap = w16_ds[b // 4][:]
            nc.sync.dma_start(w_bc[:], bass.AP(tensor=wd_ap.tensor,
                                         offset=wd_ap.offset + (b % 4) * F,
                                         ap=[[0, P], [1, F]]))
            t4 = t2.tile([P, FC, F], F16, tag="T")
            for c in range(FC):
                wtj = wT_js[b // 4]
                V.tensor_scalar(t4[:, c, :], w_bc[:],
                                wtj[:, 4 * c + b % 4:4 * c + b % 4 + 1],
                                None, ALU.add)
            sig4 = t2.tile([P, FC, F], F16, tag="sig")
            V.tensor_scalar(sig4[:].bitcast(U16), t4[:].bitcast(U16),
                            0x8000, 0x3C00, ALU.bitwise_and, ALU.bitwise_or)
            V.tensor_scalar(t4[:].bitcast(U16), t4[:].bitcast(U16),
                            0x7FFF, None, ALU.bitwise_and)
            r4 = t2.tile([P, FC, F], F16, tag="r")
            S.activation(r4[:], t4[:], AF.Sqrt)
            V.tensor_tensor(as_cache[:, :, b, :], sig4[:], r4[:], ALU.mult)
            oh = onehot[:, 31 - b:63 - b]
            for c in range(FC):
                TE.matmul(p_t32[:], oh, as_cache[:, c, b, :],
                          start=(b == 0 and c == 0),
                          stop=(b == BL - 1 and c == FC - 1))
                TE.matmul(p_d32[:], oh, r4[:, c, :],
                          start=(b == 0 and c == 0),
                          stop=(b == BL - 1 and c == FC - 1))
        t_rows = sg.tile([BL, F], F16)
        V.tensor_copy(t_rows[:], p_t32[:])
        d_rows = sg.tile([BL, F], F16)
        V.tensor_copy(d_rows[:], p_d32[:])
        p_tt = ps.tile([P, P], F16, tag="sm")
        for c in range(FC):
            TE.transpose(p_tt[:, 32 * c:32 * c + 32],
                         t_rows[:, P * c:P * c + P], i32h[:])
        tT = sg.tile([P, P], F32)
        V.tensor_copy(tT[:], p_tt[:])
        p_dd = ps.tile([P, P], F16, tag="sm")
        for c in range(FC):
            TE.transpose(p_dd[:, 32 * c:32 * c + 32],
                         d_rows[:, P * c:P * c + P], i32h[:])
        dT = sg.tile([P, P], F32)
        V.tensor_copy(dT[:], p_dd[:])

        # ---- BN1 stats + all-reduce --------------------------------------
        V.tensor_scalar(dT[:], dT[:], EPS_ROW, None, ALU.add)
        recdT = sg.tile([P, P], F32)
        V.reciprocal(recdT[:], dT[:])
        urdT = sg.tile([P, P], F32)
        V.tensor_tensor(urdT[:], uT[:], recdT[:], ALU.mult)
        z1T = sg.tile([P, P], F32)
        V.tensor_tensor(z1T[:], tT[:], urdT[:], ALU.mult)
        z1sq = t2.tile([P, P], F32, tag="r")
        V.tensor_tensor(z1sq[:], z1T[:], z1T[:], ALU.mult)
        rs = sg.tile([P, 2], F32)
        V.reduce_sum(rs[:, 0:1], z1T[:], axis=mybir.AxisListType.X)
        V.reduce_sum(rs[:, 1:2], z1sq[:], axis=mybir.AxisListType.X)
        p_s = ps.tile([1, 2], F32, tag="sm")
        TE.matmul(p_s[:], ones128[:], rs[:], start=True, stop=True)
        s_loc = sg.tile([1, 2], F32)
        V.tensor_copy(s_loc[:], p_s[:])
        cc1_in = dr.tile([1, 2], F32)
        cc1_out = dr.tile([1, 2], F32)
        nc.sync.dma_start(cc1_in[:], s_loc[:])
        if no_cc:
            nc.sync.dma_start(cc1_out[:], cc1_in[:])
        else:
            G.collective_compute("AllReduce", ALU.add,
                                 replica_groups=[list(range(NCORES))],
                                 ins=[cc1_in[:].opt()],
                                 outs=[cc1_out[:].opt()])
        sg_b = sg.tile([H, 2], F32)
        nc.sync.dma_start(sg_b[:], _bc_ap(cc1_out[:], [[0, H], [1, 2]]))

        # per-channel BN1 affine params
        mz = sg.tile([H, 1], F32)
        V.tensor_scalar(mz[:], sg_b[:, 0:1], 1.0 / NK, None, ALU.mult)
        e2m = sg.tile([H, 1], F32)
        V.tensor_scalar(e2m[:], sg_b[:, 1:2], 1.0 / NK, None, ALU.mult)
        tmp = sg.tile([H, 1], F32)
        V.tensor_tensor(tmp[:], mz[:], mz[:], ALU.mult)
        varz = sg.tile([H, 1], F32)
        V.tensor_tensor(varz[:], e2m[:], tmp[:], ALU.subtract)
        w1sq = sg.tile([H, 1], F32)
        V.tensor_tensor(w1sq[:], w1s[:], w1s[:], ALU.mult)
        var1 = sg.tile([H, 1], F32)
        V.tensor_tensor(var1[:], w1sq[:], varz[:], ALU.mult)
        invsd = sg.tile([H, 1], F32)
        S.activation(invsd[:], var1[:], AF.Ln, bias=epsb[:])
        S.activation(invsd[:], invsd[:], AF.Exp, scale=-0.5)
        alpha = sg.tile([H, 1], F32)
        V.tensor_tensor(alpha[:], w1s[:], g1s[:], ALU.mult)
        V.tensor_tensor(alpha[:], alpha[:], invsd[:], ALU.mult)
        m1 = sg.tile([H, 1], F32)
        V.tensor_tensor(m1[:], w1s[:], mz[:], ALU.mult)
        V.tensor_tensor(m1[:], m1[:], b1s[:], ALU.add)
        beta = sg.tile([H, 1], F32)
        V.tensor_tensor(beta[:], b1s[:], m1[:], ALU.subtract)
        V.tensor_tensor(beta[:], beta[:], g1s[:], ALU.mult)
        V.tensor_tensor(beta[:], beta[:], invsd[:], ALU.mult)
        V.tensor_tensor(beta[:], beta[:], be1s[:], ALU.add)

        p_ab = ps.tile([1, 2 * H], F32, tag="sm")
        TE.transpose(p_ab[:, 0:H], alpha[:], i16[:])
        TE.transpose(p_ab[:, H:2 * H], beta[:], i16[:])
        ab_row = sg.tile([1, 2 * H], F32)
        V.tensor_copy(ab_row[:], p_ab[:])
        p_abb = ps.tile([P, 2 * H], F32, tag="sm")
        TE.matmul(p_abb[:, 0:H], onesrow[:], ab_row[0:1, 0:H],
                  start=True, stop=True)
        TE.matmul(p_abb[:, H:2 * H], onesrow[:], ab_row[0:1, H:2 * H],
                  start=True, stop=True)
        abb = sg.tile([P, 2 * H], F32)
        V.tensor_copy(abb[:], p_abb[:])
        alpha_b = abb[:, 0:H]
        beta_b = abb[:, H:2 * H]

        # ---- p~ = softsign(alpha*z1+beta)*u  (fp16, [128, 128cb*16]) -----
        sfull = t2.tile([P, P, H], F16, tag="T")
        absS = t2.tile([P, P, H], F16, tag="sig")
        ptil = bigp.tile([P, P, H], F16, tag="big")
        HH = P // 2
        for h in range(2):
            sl = slice(h * HH, (h + 1) * HH)
            V.tensor_tensor(sfull[:, sl, :],
                            z1T[:, sl, None].to_broadcast([P, HH, H]),
                            alpha_b[:, None, :].to_broadcast([P, HH, H]),
                            ALU.mult)
            V.tensor_tensor(sfull[:, sl, :], sfull[:, sl, :],
                            beta_b[:, None, :].to_broadcast([P, HH, H]),
                            ALU.add)
            S.activation(absS[:, sl, :], sfull[:, sl, :], AF.Abs)
            S.activation(absS[:, sl, :], absS[:, sl, :], AF.Ln, bias=1.0)
            S.activation(absS[:, sl, :], absS[:, sl, :], AF.Exp, scale=-1.0)
            V.tensor_tensor(ptil[:, sl, :], sfull[:, sl, :], absS[:, sl, :],
                            ALU.mult)
            V.tensor_tensor(ptil[:, sl, :], ptil[:, sl, :],
                            uT[:, sl, None].to_broadcast([P, HH, H]),
                            ALU.mult)

        # ---- pass 2: GT matmuls, z2, M1/M2 -------------------------------
        z2T = sg.tile([P, FC, BL, H], F16)
        for g in range(4):
            p_gt = pgt.tile([P, FC, 8, H], F32, tag="pgt")
            for bb in range(8):
                b = 8 * g + bb
                for kc in range(FC):
                    for fc in range(FC):
                        TE.matmul(p_gt[:, kc, bb, :],
                                  as_cache[:, fc, b, P * kc:P * kc + P],
                                  ptil[:, fc * 32 + b, :],
                                  start=(fc == 0), stop=(fc == FC - 1))
            u4 = urdT[:].rearrange("p (c b) -> p c b", c=FC)
            V.tensor_tensor(
                z2T[:, :, 8 * g:8 * g + 8, :], p_gt[:],
                u4[:, :, 8 * g:8 * g + 8, None].to_broadcast([P, FC, 8, H]),
                ALU.mult)

        p_m2 = pm2.tile([H, H], F32, tag="pm2")
        p_m1 = pm2.tile([1, H], F32, tag="pm1")
        for cb in range(FC * BL):
            kc, b = divmod(cb, BL)
            TE.matmul(p_m2[:], z2T[:, kc, b, :], z2T[:, kc, b, :],
                      start=(cb == 0), stop=(cb == FC * BL - 1))
        for cb in range(FC * BL):
            kc, b = divmod(cb, BL)
            TE.matmul(p_m1[:], ones128h[:], z2T[:, kc, b, :],
                      start=(cb == 0), stop=(cb == FC * BL - 1))
        m2_sb = sg.tile([H, H], F32)
        V.tensor_copy(m2_sb[:], p_m2[:])
        m1_sb = sg.tile([1, H], F32)
        V.tensor_copy(m1_sb[:], p_m1[:])
        cc2_in = dr.tile([H + 1, H], F32)
        cc2_out = dr.tile([H + 1, H], F32)
        nc.sync.dma_start(cc2_in[0:H, :], m2_sb[:])
        nc.sync.dma_start(cc2_in[H:H + 1, :], m1_sb[:])
        if no_cc:
            nc.sync.dma_start(cc2_out[:], cc2_in[:])
        else:
            G.collective_compute("AllReduce", ALU.add,
                                 replica_groups=[list(range(NCORES))],
                                 ins=[cc2_in[:].opt()],
                                 outs=[cc2_out[:].opt()])
        m2g = sg.tile([H, H], F32)
        nc.sync.dma_start(m2g[:], cc2_out[0:H, :])
        m1_b = sg.tile([H, H], F32)
        c2ap = cc2_out[:]
        nc.sync.dma_start(m1_b[:], bass.AP(tensor=c2ap.tensor,
                                     offset=c2ap.offset + H * H,
                                     ap=[[0, H], [1, H]]))

        # ---- BN2 affine params -------------------------------------------
        p_a1 = ps.tile([H, H], F32, tag="sm")
        TE.matmul(p_a1[:], w2ts[:], m2g[:], start=True, stop=True)
        a1 = sg.tile([H, H], F32)
        V.tensor_copy(a1[:], p_a1[:])
        t16 = sg.tile([H, H], F32)
        V.tensor_tensor(t16[:], a1[:, 0:H], w2s[:], ALU.mult)
        diagq = sg.tile([H, 1], F32)
        V.reduce_sum(diagq[:], t16[:], axis=mybir.AxisListType.X)
        wm1t = sg.tile([H, H], F32)
        V.tensor_tensor(wm1t[:], w2s[:], m1_b[:], ALU.mult)
        wm1 = sg.tile([H, 1], F32)
        V.reduce_sum(wm1[:], wm1t[:], axis=mybir.AxisListType.X)
        m2o = sg.tile([H, 1], F32)
        V.tensor_scalar(m2o[:], wm1[:], 1.0 / NK, None, ALU.mult)
        V.tensor_tensor(m2o[:], m2o[:], b2s[:], ALU.add)
        eh2 = sg.tile([H, 1], F32)
        V.tensor_scalar(eh2[:], diagq[:], 1.0 / NK, None, ALU.mult)
        tb2 = sg.tile([H, 1], F32)
        V.tensor_tensor(tb2[:], b2s[:], wm1[:], ALU.mult)
        V.tensor_scalar(tb2[:], tb2[:], 2.0 / NK, None, ALU.mult)
        V.tensor_tensor(eh2[:], eh2[:], tb2[:], ALU.add)
        b2sq = sg.tile([H, 1], F32)
        V.tensor_tensor(b2sq[:], b2s[:], b2s[:], ALU.mult)
        V.tensor_tensor(eh2[:], eh2[:], b2sq[:], ALU.add)
        m2sq = sg.tile([H, 1], F32)
        V.tensor_tensor(m2sq[:], m2o[:], m2o[:], ALU.mult)
        var2 = sg.tile([H, 1], F32)
        V.tensor_tensor(var2[:], eh2[:], m2sq[:], ALU.subtract)
        invsd2 = sg.tile([H, 1], F32)
        S.activation(invsd2[:], var2[:], AF.Ln, bias=epsb[:])
        S.activation(invsd2[:], invsd2[:], AF.Exp, scale=-0.5)
        gam = sg.tile([H, 1], F32)
        V.tensor_tensor(gam[:], g2s[:], invsd2[:], ALU.mult)
        w2p = sg.tile([H, H], F16)
        V.tensor_scalar(w2p[:], w2s[:], gam[:], None, ALU.mult)
        delta = sg.tile([H, 1], F32)
        V.tensor_tensor(delta[:], b2s[:], m2o[:], ALU.subtract)
        V.tensor_tensor(delta[:], delta[:], gam[:], ALU.mult)
        V.tensor_tensor(delta[:], delta[:], be2s[:], ALU.add)

        p_w2p = ps.tile([H, H], F16, tag="sm")
        TE.transpose(p_w2p[:], w2p[:], i16h[:])
        w2pt = sg.tile([H, H], F16)
        V.tensor_copy(w2pt[:], p_w2p[:])
        bd = sg.tile([P, P], F16)
        V.memset(bd[:], 0.0)
        w2pt_d = dr.tile([H, H], F16)
        nc.sync.dma_start(w2pt_d[:], w2pt[:])
        for i in range(8):
            nc.sync.dma_start(bd[16 * i:16 * i + 16, 16 * i:16 * i + 16],
                        w2pt_d[:])
        i16big = sg.tile([H, P], F32)
        for i in range(8):
            V.tensor_copy(i16big[:, H * i:H * i + H], i16[:])
        p_dl = ps.tile([P, 1], F32, tag="sm")
        TE.matmul(p_dl[:], i16big[:], delta[:], start=True, stop=True)
        dl_rep = sg.tile([P, 1], F32)
        V.tensor_copy(dl_rep[:], p_dl[:])

        # ---- q phase + classifier ----
        qt_all = bigp.tile([P, 4, FC, P], F16, tag="big")
        qs_all = t2.tile([P, 4, F], F16, tag="T")
        for g in range(4):
            pp = pq if g % 2 == 0 else pm2
            p_z2c = pp.tile([P, F], F16, tag="pm2" if g % 2 else "pz2c",
                            name=f"pz2c{g}")
            for kc in range(FC):
                TE.transpose(p_z2c[:, P * kc:P * kc + P],
                             z2T[:, kc, 8 * g:8 * g + 8, :], i128h[:])
            z2c = wk.tile([P, F], F16, tag="z2c")
            V.tensor_copy(z2c[:], p_z2c[:])
            p_q = pp.tile([P, F], F32, tag="pm1" if g % 2 else "pqm",
                          name=f"pqm{g}")
            TE.matmul(p_q[:], bd[:], z2c[:], start=True, stop=True)
            V.tensor_scalar(qs_all[:, g, :], p_q[:], dl_rep[:], None, ALU.add)
        rq_all = t2.tile([P, 4, F], F16, tag="sig")
        q8_all = t2.tile([P, 4, F], F16, tag="r")
        for h in range(2):
            sl = slice(h * 2, (h + 1) * 2)
            S.activation(rq_all[:, sl, :], qs_all[:, sl, :], AF.Abs)
            S.activation(rq_all[:, sl, :], rq_all[:, sl, :], AF.Ln, bias=1.0)
            S.activation(rq_all[:, sl, :], rq_all[:, sl, :], AF.Exp,
                         scale=-1.0)
            V.tensor_tensor(q8_all[:, sl, :], qs_all[:, sl, :],
                            rq_all[:, sl, :], ALU.mult)
        for g in range(4):
            for kc in range(FC):
                nc.sync.dma_start_transpose(qt_all[:, g, kc, :],
                                            q8_all[:, g, P * kc:P * kc + P])
        for g in range(4):
            p_o = ps.tile([8, NCLS], F32, tag="sm")
            for o in range(H):
                for kc in range(FC):
                    jc = o * FC + kc
                    TE.matmul(p_o[:],
                              qt_all[:, g, kc, o:P:H],
                              wct_sb[:, jc, :],
                              start=(jc == 0), stop=(jc == H * FC - 1))
            out_f = wk.tile([8, NCLS], F32, tag="outf")
            V.tensor_tensor(out_f[:], p_o[:], bc_rep[:], ALU.add)
            nc.sync.dma_start(out_l[:].rearrange("(g e) n -> g e n", g=4)[g],
                        out_f[:])

    nc.finalize()
    return nc


def kernel(**inputs):
    x = np.asarray(inputs["x"], np.float32)            # [256,1,512]
    nb = np.asarray(inputs["neighbor"], np.float32)    # [256,32,1,512]
    if "prog" not in _CACHE:
        _CACHE["prog"] = build_program()
    nc = _CACHE["prog"]

    shared = {
        "att1": np.ascontiguousarray(
            np.asarray(inputs["att1_w"], np.float32)[None, :]),
        "att2": np.ascontiguousarray(
            np.asarray(inputs["att2_w"], np.float32)[None, :]),
        "w1c": np.ascontiguousarray(np.asarray(inputs["W1"], np.float32)),
        "b1": np.asarray(inputs["b1"], np.float32)[:, None].copy(),
        "g1": np.asarray(inputs["g1"], np.float32)[:, None].copy(),
        "be1": np.asarray(inputs["be1"], np.float32)[:, None].copy(),
        "w2": np.ascontiguousarray(np.asarray(inputs["W2"], np.float32)),
        "w2t": np.ascontiguousarray(np.asarray(inputs["W2"],
                                               np.float32).T),
        "b2": np.asarray(inputs["b2"], np.float32)[:, None].copy(),
        "g2": np.asarray(inputs["g2"], np.float32)[:, None].copy(),
        "be2": np.asarray(inputs["be2"], np.float32)[:, None].copy(),
        "wct": np.ascontiguousarray(
            np.asarray(inputs["Wc"], np.float32).T.astype(np.float16)),
        "bc": np.ascontiguousarray(
            np.asarray(inputs["bc"], np.float32)[None, :]),
    }
    in_maps = []
    for c in range(NCORES):
        sl = slice(c * BL, (c + 1) * BL)
        m = dict(shared)
        m["x_l"] = np.ascontiguousarray(x[sl, 0, :])
        m["nb_l"] = np.ascontiguousarray(
            nb[sl, :, 0, :].reshape(BL * N, F))
        in_maps.append(m)

    res = run_bass_kernel_spmd(nc, in_maps, core_ids=list(range(NCORES)))
    return np.concatenate([r["out_l"] for r in res.results], axis=0)



# revision 44
# speedup vs baseline: 1.2024x; 1.2024x over previous
"""TRN2 Bass kernel for nn_AttnPlainNet (gnn_message_passing).

Math (C=1 collapses everything):
  l2norm over C=1  -> u = sign(x), sgn_nb = sign(neighbor)
  att weights      -> watt[b,n] = softmax_n(s_x[b]*s_y[b,n])
  v[b,f] = sum_n watt*sgn_nb ; w = u*v
  fadj[a,e] = u_a u_e S(w_a+w_e) / (d_e + eps),  S(t)=sign(t)sqrt|t|,
  d_e = sum_a sqrt|w_a+w_e|   (A = S-matrix is symmetric)
  layer1: z1[k] = u_k t_k/(d_k+eps), t_k = sum_f S(w_f+w_k)
  BN1 is affine in z1 (stats from global z1 mean/var -> 2-float all-reduce)
  p~ = softsign(alpha*z1+beta)*u ; layer2: z2[k,c] = u_k/(d_k+eps) *
        sum_f As[f,k] p~[f,c]  (PE matmul over cached As)
  BN2 stats from z2 first/second moments (16x17 all-reduce)
  q = softsign(W2' z2 + delta) ; out = q @ WcT + bc
Sharding: pure data-parallel, 32 batches per core, 8 cores.

Implementation notes (v2):
  - one big DMA per logical input (weights host-packed) to cut HWDGE serial
    descriptor-gen time at startup
  - stage 1 softmax exp batched into one Act op so the activation-table
    sequence is exp-set -> sqrt-set -> ln/exp-set (3 loads, no thrash)
  - stage 2 per-batch element passes split across DVE (adds/sign/abs),
    Act (abs chunk + sqrt) and Pool (half the sign-apply mults)
  - t/d row sums via [128,1]-output column matmuls (free-dim 1), which
    lands them directly in the transposed layout BN1 wants
  - BN2 block-diag W2^T is host-packed; gamma/delta applied per-partition
    after the q matmul, removing the on-chip bd rebuild round trip
  - classifier runs with Wc as the stationary operand (8-wide moving), and
    the final [64,32] is PE-transposed once and stored with a single DMA
"""
from contextlib import ExitStack

import numpy as np

import concourse.bass as bass
import concourse.mybir as mybir
import concourse.tile as tile
from concourse import bacc
from concourse.bass_utils import run_bass_kernel_spmd
from concourse.masks import make_identity

# Steer the act-table-set chooser away from the partial ln-only / exp-only
# sets so Ln+Exp sequences stay resident in natural_log_exp_and_others
# (positional set ids must be preserved, so entries are emptied, not removed).
_orig_get_tables = bacc.get_activation_tables


def _patched_get_tables(arch):
    tabs = dict(_orig_get_tables(arch))
    for name in ("natural_log", "exp_and_others", "exp_and_friends"):
        if name in tabs:
            tabs[name] = set()
    return tabs


bacc.get_activation_tables = _patched_get_tables

AF = mybir.ActivationFunctionType
ALU = mybir.AluOpType
F32 = mybir.dt.float32
F16 = mybir.dt.float16
U16 = mybir.dt.uint16

B, N, F, H, NCLS = 256, 32, 512, 16, 64
NCORES = 8
BL = B // NCORES          # 32 local batches
FC = 4                    # f/k chunks of 128
P = 128
EPS_ROW = 1e-7
EPS_BN = 1e-5
NK = float(B * F)         # BN normalizer (global)
NJ = N * BL // P          # 8 neighbor tiles of 128 rows

# wpk column layout (host-packed [H, 42] f32)
C_W1, C_B1, C_G1, C_BE1 = 0, 1, 2, 3
C_W2 = 4            # 4:20   W2[i, j]
C_W2T = 20          # 20:36  W2[j, i]
C_B2, C_G2, C_BE2 = 36, 37, 38
C_W1SQ, C_W1G1, C_B2SQ = 39, 40, 41
WPKC = 42

_CACHE = {}


def _bc_ap(handle_ap, ap, extra_off=0):
    """AP with explicit [stride, count] dims over a tensor handle's AP."""
    return bass.AP(tensor=handle_ap.tensor, offset=handle_ap.offset + extra_off,
                   ap=ap)


def build_program(no_cc=False, debug=False):
    nc = bacc.Bacc("TRN2", num_devices=NCORES)

    # ---- I/O -------------------------------------------------------------
    x_l = nc.dram_tensor("x_l", [BL, F], F32, kind="ExternalInput")
    nb_l = nc.dram_tensor("nb_l", [BL * N, F], F32, kind="ExternalInput")
    attp = nc.dram_tensor("attp", [2, F], F32, kind="ExternalInput")
    wpk = nc.dram_tensor("wpk", [H, WPKC], F32, kind="ExternalInput")
    wbig = nc.dram_tensor("wbig", [P, H * F // P * NCLS + P + NJ * 32], F16,
                          kind="ExternalInput")
    bcc = nc.dram_tensor("bcc", [NCLS, 1], F32, kind="ExternalInput")
    cst = nc.dram_tensor("cst", [32, NJ + P], F32, kind="ExternalInput")
    out_l = nc.dram_tensor("out_l", [BL, NCLS], F32, kind="ExternalOutput")
    if debug:
        dbg_w = nc.dram_tensor("dbg_w", [32, F], F16, kind="ExternalOutput")
        dbg_tT = nc.dram_tensor("dbg_tT", [P, P], F32, kind="ExternalOutput")
        dbg_dT = nc.dram_tensor("dbg_dT", [P, P], F32, kind="ExternalOutput")
        dbg_z1T = nc.dram_tensor("dbg_z1T", [P, P], F32,
                                 kind="ExternalOutput")
        dbg_as0 = nc.dram_tensor("dbg_as0", [P, FC, F], F16,
                                 kind="ExternalOutput")
        dbg_ab = nc.dram_tensor("dbg_ab", [P, 2 * H], F32,
                                kind="ExternalOutput")
        dbg_z2T = nc.dram_tensor("dbg_z2T", [P, FC, BL, H], F16,
                                 kind="ExternalOutput")
        dbg_ptil = nc.dram_tensor("dbg_ptil", [P, P, H], F16,
                                  kind="ExternalOutput")
        dbg_qs = nc.dram_tensor("dbg_qs", [P, 4, F], F16,
                                kind="ExternalOutput")

    with tile.TileContext(nc) as tc, ExitStack() as ctx:
        sg = ctx.enter_context(tc.tile_pool(name="singles", bufs=1))
        wk = ctx.enter_context(tc.tile_pool(name="work", bufs=2))
        tl = ctx.enter_context(tc.tile_pool(name="tail", bufs=2))
        dr = ctx.enter_context(tc.tile_pool(name="dram", bufs=1, space="DRAM"))
        s1ctx = ExitStack()
        s1 = s1ctx.enter_context(tc.tile_pool(name="stage1", bufs=1))
        s1w = s1ctx.enter_context(tc.tile_pool(name="s1work", bufs=3))
        p1 = s1ctx.enter_context(tc.tile_pool(name="psum1", bufs=1,
                                              space="PSUM"))


        V, S, G = nc.vector, nc.scalar, nc.gpsimd
        TE = nc.tensor

        # ---- startup DMAs (priority order: x, att, neighbors, weights) ---
        xsb = s1.tile([BL, F], F32)
        nc.sync.dma_start(xsb[:], x_l[:])
        attsb2 = s1.tile([1, F], F32)
        nc.sync.dma_start(attsb2[:], attp[1:2, :])
        nb_r = nb_l[:].rearrange("(j p) f -> j p f", p=P)
        wpks = sg.tile([H, WPKC], F32)
        nc.sync.dma_start(wpks[:], wpk[:])
        wbigs = sg.tile([P, H * F // P * NCLS + P + NJ * 32], F16)
        nc.sync.dma_start(wbigs[:], wbig[:])
        bccs = sg.tile([NCLS, 1], F32)
        nc.sync.dma_start(bccs[:], bcc[:])
        csts = sg.tile([32, NJ + P], F32)
        nc.sync.dma_start(csts[:], cst[:])
        wct_sb = wbigs[:, 0:H * F // P * NCLS].rearrange(
            "p (jc n) -> p jc n", n=NCLS)
        bd0 = wbigs[:, H * F // P * NCLS:H * F // P * NCLS + P]
        mask8 = wbigs[:, H * F // P * NCLS + P:].rearrange(
            "p (j c) -> p j c", c=32)
        jmask = csts[:, 0:NJ]
        a32 = csts[:, NJ:]

        # ---- constants ---------------------------------------------------
        i32h = sg.tile([32, 32], F16)
        make_identity(nc, i32h[:])
        i128h = sg.tile([P, P], F16)
        make_identity(nc, i128h[:])
        i16f = sg.tile([H, H], F32)
        make_identity(nc, i16f[:])
        i64f = sg.tile([NCLS, NCLS], F32)
        make_identity(nc, i64f[:])
        i16big = sg.tile([H, P], F32)
        for i in range(8):
            V.tensor_copy(i16big[:, H * i:H * i + H], i16f[:])
        ones128 = sg.tile([P, 1], F32)
        V.memset(ones128[:], 1.0)
        ones128h = sg.tile([P, 1], F16)
        V.memset(ones128h[:], 1.0)
        onesrow = sg.tile([1, P], F32)
        V.memset(onesrow[:], 1.0)
        epsb = sg.tile([H, 1], F32)
        V.memset(epsb[:], EPS_BN)

        # ---- stage 0: u = sign(x), s_x, broadcast helpers ----------------
        u16 = s1.tile([BL, F], F16)
        S.activation(u16[:], xsb[:], AF.Sign)
        p_att2 = p1.tile([P, F], F32)
        TE.matmul(p_att2[:], onesrow[:], attsb2[:], start=True, stop=True)
        att2_b = s1.tile([P, F], F32)
        S.activation(att2_b[:], p_att2[:], AF.Copy)
        # uT16[p, (c b)] = u[b, 128c+p]
        p_ut = p1.tile([P, FC, 32], F16, tag="ptr")
        for c in range(FC):
            TE.transpose(p_ut[:, c, :], u16[:, P * c:P * c + P], i32h[:])
        uT16 = sg.tile([P, P], F16)
        V.tensor_copy(uT16[:].rearrange("p (c b) -> p c b", c=FC), p_ut[:])
        # s_x[b] = sum_f u[b,f] att1[f] via PE over transposed u
        att1c = s1.tile([P, FC], F32)
        nc.sync.dma_start(att1c[:], _bc_ap(attp[:], [[1, P], [P, FC]]))
        att1c16 = s1.tile([P, FC], F16)
        V.tensor_copy(att1c16[:], att1c[:])
        p_sxc = p1.tile([32, 1], F32, tag="pcol")
        for c in range(FC):
            TE.matmul(p_sxc[:], uT16[:, 32 * c:32 * c + 32],
                      att1c16[:, c:c + 1], start=(c == 0), stop=(c == FC - 1))
        sx_col = s1.tile([BL, 1], F32)
        V.tensor_copy(sx_col[:], p_sxc[:])
        sxm = s1.tile([32, NJ], F32)
        V.tensor_tensor(sxm[:], sx_col[:].to_broadcast([32, NJ]), jmask,
                        ALU.mult)
        p_sx = p1.tile([P, NJ], F32, tag="pbig")
        TE.matmul(p_sx[:], a32, sxm[:], start=True, stop=True)
        sx_rep = s1.tile([P, NJ], F32)
        V.tensor_copy(sx_rep[:], p_sx[:])

        # ---- stage 1: neighbor signs, logits, softmax, v -----------------
        sy8 = s1.tile([P, NJ], F32)
        lcol8 = s1.tile([P, NJ], F32)
        ecol8 = s1.tile([P, NJ], F32)
        p_dn = p1.tile([32, 1], F32, tag="pcol")
        p_v = p1.tile([32, F], F32, tag="pbig")
        for j in range(NJ):
            nbt = s1w.tile([P, F], F32, tag="nbt", bufs=2)
            nc.sync.dma_start(nbt[:], nb_r[j])
            sgn = s1w.tile([P, F], F16, tag="sgn")
            S.activation(sgn[:], nbt[:], AF.Sign)
            sydump = s1w.tile([P, F], F16, tag="sydump", bufs=2)
            V.scalar_tensor_tensor(sydump[:], sgn[:], 0.0,
                                   att2_b[:], ALU.bypass, ALU.mult,
                                   accum_out=sy8[:, j:j + 1])
            V.tensor_tensor(lcol8[:, j:j + 1], sy8[:, j:j + 1],
                            sx_rep[:, j:j + 1], ALU.mult)
            S.activation(ecol8[:, j:j + 1], lcol8[:, j:j + 1], AF.Exp)
            wd32 = s1w.tile([P, 32], F16, tag="wd32")
            V.tensor_tensor(wd32[:], ecol8[:, j:j + 1].to_broadcast([P, 32]),
                            mask8[:, j, :], ALU.mult)  # mask8 is f16 view
            TE.matmul(p_dn[:], wd32[:], ones128h[:],
                      start=(j == 0), stop=(j == NJ - 1))
            TE.matmul(p_v[:], wd32[:], sgn[:],
                      start=(j == 0), stop=(j == NJ - 1))
        rdn32 = s1.tile([32, 1], F32)
        V.reciprocal(rdn32[:], p_dn[:])
        # w16 = (v * 1/dn) * u  in one pass
        w16_all = s1.tile([32, F], F16)
        V.scalar_tensor_tensor(w16_all[:], p_v[:], rdn32[:], u16[:],
                               ALU.mult, ALU.mult)
        w16_d = dr.tile([32, F], F16)
        nc.sync.dma_start(w16_d[:], w16_all[:])
        if debug:
            nc.sync.dma_start(dbg_w[:], w16_all[:])
        # wT_all[p, c, b] = w[b, 128c+p]
        p_wt = p1.tile([P, FC, 32], F16, tag="ptr")
        for c in range(FC):
            TE.transpose(p_wt[:, c, :], w16_all[:, P * c:P * c + P], i32h[:])
        wT_all = sg.tile([P, FC, 32], F32)
        V.tensor_copy(wT_all[:], p_wt[:])
        s1ctx.close()

        # ---- stage 2: A matrix, t/d column sums, As cache ----------------
        ps2ctx = ExitStack()
        ps2 = ps2ctx.enter_context(tc.tile_pool(name="psum2", bufs=1,
                                                space="PSUM"))
        as_cache = sg.tile([P, FC, BL, F], F16)
        pt = ps2.tile([P, FC, BL], F32)
        pd = ps2.tile([P, FC, BL], F32)
        for b in range(BL):
            w_bc = wk.tile([P, F], F16, tag="wbc")
            nc.sync.dma_start(w_bc[:], _bc_ap(w16_d[:], [[0, P], [1, F]],
                                              extra_off=b * F))
            t4 = wk.tile([P, FC, F], F16, tag="t4")
            for c in range(FC):
                V.tensor_scalar(t4[:, c, :], w_bc[:], wT_all[:, c, b:b + 1],
                                None, ALU.add)
            asb = as_cache[:, :, b, :]
            V.tensor_scalar(asb.bitcast(U16), t4[:].bitcast(U16),
                            0x8000, 0x3C00, ALU.bitwise_and, ALU.bitwise_or)
            # |t4| in place: chunks 0-2 on DVE, chunk 3 on Act
            for c in range(3):
                V.tensor_scalar(t4[:, c, :].bitcast(U16),
                                t4[:, c, :].bitcast(U16),
                                0x7FFF, None, ALU.bitwise_and)
            S.activation(t4[:, 3, :], t4[:, 3, :], AF.Abs)
            r4 = wk.tile([P, FC, F], F16, tag="r4")
            S.activation(r4[:], t4[:], AF.Sqrt)
            for c in range(2):
                V.tensor_tensor(as_cache[:, c, b, :], as_cache[:, c, b, :],
                                r4[:, c, :], ALU.mult)
            for c in range(2, FC):
                G.tensor_tensor(as_cache[:, c, b, :], as_cache[:, c, b, :],
                                r4[:, c, :], ALU.mult)
            for kc in range(FC):
                for c in range(FC):
                    TE.matmul(pt[:, kc, b:b + 1],
                              as_cache[:, c, b, P * kc:P * kc + P],
                              ones128h[:], start=(c == 0), stop=(c == FC - 1))
                    TE.matmul(pd[:, kc, b:b + 1],
                              r4[:, c, P * kc:P * kc + P],
                              ones128h[:], start=(c == 0), stop=(c == FC - 1))

        # ---- BN1 ---------------------------------------------------------
        tT = sg.tile([P, P], F32)
        V.tensor_copy(tT[:].rearrange("p (c b) -> p c b", c=FC), pt[:])
        dT = sg.tile([P, P], F32)
        V.tensor_copy(dT[:].rearrange("p (c b) -> p c b", c=FC), pd[:])
        ps2ctx.close()
        V.tensor_scalar(dT[:], dT[:], EPS_ROW, None, ALU.add)
        recdT = sg.tile([P, P], F32)
        V.reciprocal(recdT[:], dT[:])
        urdT = sg.tile([P, P], F32)
        V.tensor_tensor(urdT[:], uT16[:], recdT[:], ALU.mult)
        z1T = sg.tile([P, P], F32)
        V.tensor_tensor(z1T[:], tT[:], urdT[:], ALU.mult)
        if debug:
            nc.sync.dma_start(dbg_tT[:], tT[:])
            nc.sync.dma_start(dbg_dT[:], dT[:])
            nc.sync.dma_start(dbg_z1T[:], z1T[:])
            nc.sync.dma_start(dbg_as0[:], as_cache[:, :, 0, :])
        z1sq = tl.tile([P, P], F32, tag="z1sq")
        V.tensor_tensor(z1sq[:], z1T[:], z1T[:], ALU.mult)
        rs = tl.tile([P, 2], F32, tag="rs")
        V.reduce_sum(rs[:, 0:1], z1T[:], axis=mybir.AxisListType.X)
        V.reduce_sum(rs[:, 1:2], z1sq[:], axis=mybir.AxisListType.X)
        psm = ctx.enter_context(tc.tile_pool(name="psmall", bufs=1,
                                             space="PSUM"))
        p_s = psm.tile([1, 2], F32, tag="sma")
        TE.matmul(p_s[:], ones128[:], rs[:], start=True, stop=True)
        s_loc = sg.tile([1, 2], F32)
        V.tensor_copy(s_loc[:], p_s[:])
        cc1_in = dr.tile([1, 2], F32)
        cc1_out = dr.tile([1, 2], F32)
        nc.sync.dma_start(cc1_in[:], s_loc[:])
        if no_cc:
            nc.sync.dma_start(cc1_out[:], cc1_in[:])
        else:
            G.collective_compute("AllReduce", ALU.add,
                                 replica_groups=[list(range(NCORES))],
                                 ins=[cc1_in[:].opt()],
                                 outs=[cc1_out[:].opt()])
        s2 = sg.tile([1, 2], F32)
        nc.sync.dma_start(s2[:], cc1_out[:])
        p_sg = psm.tile([H, 2], F32, tag="sma")
        TE.matmul(p_sg[:], onesrow[0:1, 0:H], s2[:], start=True, stop=True)
        sgb = sg.tile([H, 2], F32)
        V.tensor_copy(sgb[:], p_sg[:])

        # per-channel BN1 affine params: alpha = w1 g1 invsd,
        # beta = be1 - alpha * mz   (written as alpha*(-mz) + be1)
        mzneg = sg.tile([H, 1], F32)
        V.tensor_scalar(mzneg[:], sgb[:, 0:1], -1.0 / NK, None, ALU.mult)
        e2m = sg.tile([H, 1], F32)
        V.tensor_scalar(e2m[:], sgb[:, 1:2], 1.0 / NK, None, ALU.mult)
        mz2 = sg.tile([H, 1], F32)
        V.tensor_tensor(mz2[:], mzneg[:], mzneg[:], ALU.mult)
        varz = sg.tile([H, 1], F32)
        V.tensor_tensor(varz[:], e2m[:], mz2[:], ALU.subtract)
        var1 = sg.tile([H, 1], F32)
        V.tensor_tensor(var1[:], wpks[:, C_W1SQ:C_W1SQ + 1], varz[:],
                        ALU.mult)
        invsd = sg.tile([H, 1], F32)
        S.activation(invsd[:], var1[:], AF.Ln, bias=epsb[:])
        S.activation(invsd[:], invsd[:], AF.Exp, scale=-0.5)
        ab2 = sg.tile([H, 2], F32)
        V.tensor_tensor(ab2[:, 0:1], wpks[:, C_W1G1:C_W1G1 + 1], invsd[:],
                        ALU.mult)
        V.scalar_tensor_tensor(ab2[:, 1:2], ab2[:, 0:1], mzneg[:],
                               wpks[:, C_BE1:C_BE1 + 1], ALU.mult, ALU.add)
        p_ab = psm.tile([1, 2 * H], F32, tag="sma")
        TE.transpose(p_ab[:, 0:H], ab2[:, 0:1], i16f[:])
        TE.transpose(p_ab[:, H:2 * H], ab2[:, 1:2], i16f[:])
        abrow = sg.tile([1, 2 * H], F32)
        V.tensor_copy(abrow[:], p_ab[:])
        p_abb = psm.tile([P, 2 * H], F32, tag="smb")
        TE.matmul(p_abb[:], onesrow[:], abrow[:], start=True, stop=True)
        abb = sg.tile([P, 2 * H], F32)
        V.tensor_copy(abb[:], p_abb[:])
        if debug:
            nc.sync.dma_start(dbg_ab[:], abb[:])
        alpha_b = abb[:, 0:H]
        beta_b = abb[:, H:2 * H]

        # ---- p~ chunks + GT matmuls (interleaved) ------------------------
        midctx = ExitStack()
        mid = midctx.enter_context(tc.tile_pool(name="mid", bufs=1))
        pgtctx = ExitStack()
        pgt = pgtctx.enter_context(tc.tile_pool(name="pgt", bufs=1,
                                                space="PSUM"))
        ptil = mid.tile([P, P, H], F16)
        p_gt = pgt.tile([P, 4, FC, 8, H], F32)
        for c in range(FC):
            sl = slice(32 * c, 32 * c + 32)
            sfull = tl.tile([P, 32, H], F16, tag="sfull")
            V.tensor_tensor(sfull[:],
                            z1T[:, sl, None].to_broadcast([P, 32, H]),
                            alpha_b[:, None, :].to_broadcast([P, 32, H]),
                            ALU.mult)
            V.tensor_tensor(sfull[:], sfull[:],
                            beta_b[:, None, :].to_broadcast([P, 32, H]),
                            ALU.add)
            absS = tl.tile([P, 32, H], F16, tag="absS")
            S.activation(absS[:], sfull[:], AF.Abs)
            S.activation(absS[:], absS[:], AF.Ln, bias=1.0)
            S.activation(absS[:], absS[:], AF.Exp, scale=-1.0)
            V.tensor_tensor(ptil[:, sl, :], sfull[:], absS[:], ALU.mult)
            G.tensor_tensor(ptil[:, sl, :], ptil[:, sl, :],
                            uT16[:, sl, None].to_broadcast([P, 32, H]),
                            ALU.mult)
        # accumulation groups must be consecutive within a PSUM tile, so the
        # f-chunk loop is innermost (full ptil required before GT starts)
        for g in range(4):
            for bb in range(8):
                b = 8 * g + bb
                for kc in range(FC):
                    for c in range(FC):
                        TE.matmul(p_gt[:, g, kc, bb, :],
                                  as_cache[:, c, b, P * kc:P * kc + P],
                                  ptil[:, 32 * c + b, :],
                                  start=(c == 0), stop=(c == FC - 1))

        # ---- z2, BN2 moments ---------------------------------------------
        z2T = sg.tile([P, FC, BL, H], F16)
        u4 = urdT[:].rearrange("p (c b) -> p c b", c=FC)
        for g in range(4):
            V.tensor_tensor(
                z2T[:, :, 8 * g:8 * g + 8, :], p_gt[:, g],
                u4[:, :, 8 * g:8 * g + 8, None].to_broadcast([P, FC, 8, H]),
                ALU.mult)
        if debug:
            nc.sync.dma_start(dbg_z2T[:], z2T[:])
            nc.sync.dma_start(dbg_ptil[:], ptil[:])
        p_m2 = psm.tile([H, H], F32, tag="sma")
        p_m1 = psm.tile([1, H], F32, tag="smb")
        for cb in range(FC * BL):
            kc, b = divmod(cb, BL)
            TE.matmul(p_m2[:], z2T[:, kc, b, :], z2T[:, kc, b, :],
                      start=(cb == 0), stop=(cb == FC * BL - 1))
        for cb in range(FC * BL):
            kc, b = divmod(cb, BL)
            TE.matmul(p_m1[:], ones128h[:], z2T[:, kc, b, :],
                      start=(cb == 0), stop=(cb == FC * BL - 1))
        pgtctx.close()
        midctx.close()
        m2_sb = sg.tile([H, H], F32)
        V.tensor_copy(m2_sb[:], p_m2[:])
        m1_sb = sg.tile([1, H], F32)
        V.tensor_copy(m1_sb[:], p_m1[:])
        cc2_in = dr.tile([H + 1, H], F32)
        cc2_out = dr.tile([H + 1, H], F32)
        nc.sync.dma_start(cc2_in[0:H, :], m2_sb[:])
        nc.sync.dma_start(cc2_in[H:H + 1, :], m1_sb[:])
        if no_cc:
            nc.sync.dma_start(cc2_out[:], cc2_in[:])
        else:
            G.collective_compute("AllReduce", ALU.add,
                                 replica_groups=[list(range(NCORES))],
                                 ins=[cc2_in[:].opt()],
                                 outs=[cc2_out[:].opt()])
        m2g = sg.tile([H, H], F32)
        nc.sync.dma_start(m2g[:], cc2_out[0:H, :])
        m1col = sg.tile([H, 1], F32)
        nc.sync.dma_start(m1col[:], _bc_ap(cc2_out[:], [[1, H], [1, 1]],
                                           extra_off=H * H))

        # ---- BN2 affine params -------------------------------------------
        w2s = wpks[:, C_W2:C_W2 + H]
        w2ts = wpks[:, C_W2T:C_W2T + H]
        b2col = wpks[:, C_B2:C_B2 + 1]
        p_a1 = psm.tile([H, H], F32, tag="sma")
        TE.matmul(p_a1[:], w2ts, m2g[:], start=True, stop=True)
        a1 = sg.tile([H, H], F32)
        V.tensor_copy(a1[:], p_a1[:])
        t16 = sg.tile([H, H], F32)
        V.tensor_tensor(t16[:], a1[:], w2s, ALU.mult)
        diagq = sg.tile([H, 1], F32)
        V.reduce_sum(diagq[:], t16[:], axis=mybir.AxisListType.X)
        p_wm1 = psm.tile([H, 1], F32, tag="smb")
        TE.matmul(p_wm1[:], w2ts, m1col[:], start=True, stop=True)
        wm1 = sg.tile([H, 1], F32)
        V.tensor_copy(wm1[:], p_wm1[:])
        m2o = sg.tile([H, 1], F32)
        V.tensor_scalar(m2o[:], wm1[:], 1.0 / NK, b2col, ALU.mult, ALU.add)
        tb2 = sg.tile([H, 1], F32)
        V.scalar_tensor_tensor(tb2[:], wm1[:], 2.0 / NK, b2col,
                               ALU.mult, ALU.mult)
        eh2 = sg.tile([H, 1], F32)
        V.tensor_scalar(eh2[:], diagq[:], 1.0 / NK, None, ALU.mult)
        V.tensor_tensor(eh2[:], eh2[:], tb2[:], ALU.add)
        V.tensor_tensor(eh2[:], eh2[:], wpks[:, C_B2SQ:C_B2SQ + 1], ALU.add)
        m2sq = sg.tile([H, 1], F32)
        V.tensor_tensor(m2sq[:], m2o[:], m2o[:], ALU.mult)
        var2 = sg.tile([H, 1], F32)
        V.tensor_tensor(var2[:], eh2[:], m2sq[:], ALU.subtract)
        invsd2 = sg.tile([H, 1], F32)
        S.activation(invsd2[:], var2[:], AF.Ln, bias=epsb[:])
        S.activation(invsd2[:], invsd2[:], AF.Exp, scale=-0.5)
        gd2 = sg.tile([H, 2], F32)
        V.tensor_tensor(gd2[:, 0:1], wpks[:, C_G2:C_G2 + 1], invsd2[:],
                        ALU.mult)
        d0 = sg.tile([H, 1], F32)
        V.tensor_tensor(d0[:], b2col, m2o[:], ALU.subtract)
        V.scalar_tensor_tensor(gd2[:, 1:2], d0[:], gd2[:, 0:1],
                               wpks[:, C_BE2:C_BE2 + 1], ALU.mult, ALU.add)
        p_gd = psm.tile([P, 2], F32, tag="sma")
        TE.matmul(p_gd[:], i16big[:], gd2[:], start=True, stop=True)
        gdrep = sg.tile([P, 2], F32)
        V.tensor_copy(gdrep[:], p_gd[:])

        # ---- q phase + classifier ----------------------------------------
        pz = ctx.enter_context(tc.tile_pool(name="pz", bufs=2,
                                            space="PSUM"))
        qs_all = sg.tile([P, 4, F], F16)
        q8_all = sg.tile([P, 4, F], F16)
        for g in range(4):
            p_z2c = pz.tile([P, F], F16, tag="pz2c")
            for kc in range(FC):
                TE.transpose(p_z2c[:, P * kc:P * kc + P],
                             z2T[:, kc, 8 * g:8 * g + 8, :], i128h[:])
            z2c = tl.tile([P, F], F16, tag="z2c")
            if g % 2 == 0:
                V.tensor_copy(z2c[:], p_z2c[:])
            else:
                S.activation(z2c[:], p_z2c[:], AF.Copy)
            p_q = pz.tile([P, F], F32, tag="pq")
            TE.matmul(p_q[:], bd0, z2c[:], start=True, stop=True)
            V.tensor_scalar(qs_all[:, g, :], p_q[:], gdrep[:, 0:1],
                            gdrep[:, 1:2], ALU.mult, ALU.add)
        if debug:
            nc.sync.dma_start(dbg_qs[:], qs_all[:])
        for h in range(2):
            sl = slice(2 * h, 2 * h + 2)
            rq = tl.tile([P, 2, F], F16, tag="rq")
            S.activation(rq[:], qs_all[:, sl, :], AF.Abs)
            S.activation(rq[:], rq[:], AF.Ln, bias=1.0)
            S.activation(rq[:], rq[:], AF.Exp, scale=-1.0)
            V.tensor_tensor(q8_all[:, sl, :], qs_all[:, sl, :], rq[:],
                            ALU.mult)
        p_oT = psm.tile([NCLS, 4, 8], F32, tag="smb")
        for g in range(4):
            for kc in range(FC):
                p_qt = pz.tile([P, P], F16, tag="pqt")
                TE.transpose(p_qt[:], q8_all[:, g, P * kc:P * kc + P],
                             i128h[:])
                qt = tl.tile([P, P], F16, tag="qt")
                if kc % 2 == 0:
                    V.tensor_copy(qt[:], p_qt[:])
                else:
                    S.activation(qt[:], p_qt[:], AF.Copy)
                for o in range(H):
                    jc = o * FC + kc
                    TE.matmul(p_oT[:, g, :], wct_sb[:, jc, :],
                              qt[:, o:P:H],
                              start=(kc == 0 and o == 0),
                              stop=(kc == FC - 1 and o == H - 1))
        outT = sg.tile([NCLS, 4, 8], F32)
        V.tensor_scalar(outT[:], p_oT[:], bccs[:], None, ALU.add)
        p_out = psm.tile([BL, NCLS], F32, tag="sma")
        TE.transpose(p_out[:], outT[:].rearrange("n g e -> n (g e)"), i64f[:])
        out_f = sg.tile([BL, NCLS], F32)
        V.tensor_copy(out_f[:], p_out[:])
        nc.sync.dma_start(out_l[:], out_f[:])

    nc.finalize()
    return nc


def kernel(**inputs):
    x = np.asarray(inputs["x"], np.float32)            # [256,1,512]
    nb = np.asarray(inputs["neighbor"], np.float32)    # [256,32,1,512]
    if "prog" not in _CACHE:
        _CACHE["prog"] = build_program()
    nc = _CACHE["prog"]

    w1 = np.asarray(inputs["W1"], np.float32).reshape(H)
    b1 = np.asarray(inputs["b1"], np.float32)
    g1 = np.asarray(inputs["g1"], np.float32)
    be1 = np.asarray(inputs["be1"], np.float32)
    w2 = np.asarray(inputs["W2"], np.float32)
    b2 = np.asarray(inputs["b2"], np.float32)
    g2 = np.asarray(inputs["g2"], np.float32)
    be2 = np.asarray(inputs["be2"], np.float32)
    wc = np.asarray(inputs["Wc"], np.float32)
    bc = np.asarray(inputs["bc"], np.float32)

    wpk = np.zeros((H, WPKC), np.float32)
    wpk[:, C_W1] = w1
    wpk[:, C_B1] = b1
    wpk[:, C_G1] = g1
    wpk[:, C_BE1] = be1
    wpk[:, C_W2:C_W2 + H] = w2
    wpk[:, C_W2T:C_W2T + H] = w2.T
    wpk[:, C_B2] = b2
    wpk[:, C_G2] = g2
    wpk[:, C_BE2] = be2
    wpk[:, C_W1SQ] = w1 * w1
    wpk[:, C_W1G1] = w1 * g1
    wpk[:, C_B2SQ] = b2 * b2

    # wct packed [p, jc*NCLS + n] = WcT[jc*128 + p, n]; bd0 = blockdiag(W2T)
    wct = wc.T.astype(np.float16)                      # [H*F, NCLS]
    wctp = wct.reshape(H * F // P, P, NCLS).transpose(1, 0, 2).reshape(
        P, H * F // P * NCLS)
    bd0 = np.zeros((P, P), np.float16)
    w2t16 = w2.T.astype(np.float16)
    for i in range(8):
        bd0[16 * i:16 * i + 16, 16 * i:16 * i + 16] = w2t16
    # mask8[p, j, c] = 1 iff c == 4j + p//32  (softmax block selector)
    pidx = np.arange(P)[:, None, None]
    jidx = np.arange(NJ)[None, :, None]
    cidx = np.arange(32)[None, None, :]
    mask8 = (cidx == 4 * jidx + pidx // 32).astype(np.float16).reshape(
        P, NJ * 32)
    wbig = np.ascontiguousarray(np.concatenate([wctp, bd0, mask8], axis=1))
    # cst: [32, NJ + 128]: jmask[b, j] = (b//4 == j); a32[b, p] = (b%4 == p//32)
    bidx = np.arange(32)[:, None]
    jmask = (bidx // 4 == np.arange(NJ)[None, :]).astype(np.float32)
    a32 = (bidx % 4 == np.arange(P)[None, :] // 32).astype(np.float32)
    cstm = np.ascontiguousarray(np.concatenate([jmask, a32], axis=1))

    shared = {
        "attp": np.ascontiguousarray(np.stack([
            np.asarray(inputs["att1_w"], np.float32),
            np.asarray(inputs["att2_w"], np.float32)])),
        "wpk": wpk,
        "wbig": wbig,
        "bcc": np.ascontiguousarray(bc[:, None]),
        "cst": cstm,
    }
    in_maps = []
    for c in range(NCORES):
        sl = slice(c * BL, (c + 1) * BL)
        m = dict(shared)
        m["x_l"] = np.ascontiguousarray(x[sl, 0, :])
        m["nb_l"] = np.ascontiguousarray(
            nb[sl, :, 0, :].reshape(BL * N, F))
        in_maps.append(m)

    res = run_bass_kernel_spmd(nc, in_maps, core_ids=list(range(NCORES)))
    return np.concatenate([r["out_l"] for r in res.results], axis=0)


# revision 54
# speedup vs baseline: 1.2836x; 1.0675x over previous
"""TRN2 Bass kernel for nn_AttnPlainNet (gnn_message_passing).

Math (C=1 collapses everything):
  l2norm over C=1  -> u = sign(x), sgn_nb = sign(neighbor)
  att weights      -> watt[b,n] = softmax_n(s_x[b]*s_y[b,n])
  v[b,f] = sum_n watt*sgn_nb ; w = u*v
  fadj[a,e] = u_a u_e S(w_a+w_e) / (d_e + eps),  S(t)=sign(t)sqrt|t|,
  d_e = sum_a sqrt|w_a+w_e|   (A = S-matrix is symmetric)
  layer1: z1[k] = u_k t_k/(d_k+eps), t_k = sum_f S(w_f+w_k)
  BN1 is affine in z1 (stats from global z1 mean/var -> 2-float all-reduce)
  p~ = softsign(alpha*z1+beta)*u ; layer2: z2[k,c] = u_k/(d_k+eps) *
        sum_f As[f,k] p~[f,c]  (PE matmul over cached As)
  BN2 stats from z2 first/second moments (16x17 all-reduce)
  q = softsign(W2' z2 + delta) ; out = q @ WcT + bc
Sharding: pure data-parallel, 32 batches per core, 8 cores.

Implementation notes (v2):
  - one big DMA per logical input (weights host-packed) to cut HWDGE serial
    descriptor-gen time at startup
  - stage 1 softmax exp batched into one Act op so the activation-table
    sequence is exp-set -> sqrt-set -> ln/exp-set (3 loads, no thrash)
  - stage 2 per-batch element passes split across DVE (adds/sign/abs),
    Act (abs chunk + sqrt) and Pool (half the sign-apply mults)
  - t/d row sums via [128,1]-output column matmuls (free-dim 1), which
    lands them directly in the transposed layout BN1 wants
  - BN2 block-diag W2^T is host-packed; gamma/delta applied per-partition
    after the q matmul, removing the on-chip bd rebuild round trip
  - classifier runs with Wc as the stationary operand (8-wide moving), and
    the final [64,32] is PE-transposed once and stored with a single DMA
"""
from contextlib import ExitStack

import numpy as np

import concourse.bass as bass
import concourse.mybir as mybir
import concourse.tile as tile
from concourse import bacc
from concourse.bass_utils import run_bass_kernel_spmd
from concourse.masks import make_identity

# Steer the act-table-set chooser away from the partial ln-only / exp-only
# sets so Ln+Exp sequences stay resident in natural_log_exp_and_others
# (positional set ids must be preserved, so entries are emptied, not removed).
_orig_get_tables = bacc.get_activation_tables


def _patched_get_tables(arch):
    tabs = dict(_orig_get_tables(arch))
    for name in ("natural_log", "exp_and_others", "exp_and_friends"):
        if name in tabs:
            tabs[name] = set()
    return tabs


bacc.get_activation_tables = _patched_get_tables

AF = mybir.ActivationFunctionType
ALU = mybir.AluOpType
F32 = mybir.dt.float32
F16 = mybir.dt.float16
U16 = mybir.dt.uint16

B, N, F, H, NCLS = 256, 32, 512, 16, 64
NCORES = 8
BL = B // NCORES          # 32 local batches
FC = 4                    # f/k chunks of 128
P = 128
EPS_ROW = 1e-7
EPS_BN = 1e-5
NK = float(B * F)         # BN normalizer (global)
NJ = N * BL // P          # 8 neighbor tiles of 128 rows

# wpk column layout (host-packed [H, 42] f32)
C_W1, C_B1, C_G1, C_BE1 = 0, 1, 2, 3
C_W2 = 4            # 4:20   W2[i, j]
C_W2T = 20          # 20:36  W2[j, i]
C_B2, C_G2, C_BE2 = 36, 37, 38
C_W1SQ, C_W1G1, C_B2SQ = 39, 40, 41
WPKC = 42

_CACHE = {}


def _bc_ap(handle_ap, ap, extra_off=0):
    """AP with explicit [stride, count] dims over a tensor handle's AP."""
    return bass.AP(tensor=handle_ap.tensor, offset=handle_ap.offset + extra_off,
                   ap=ap)


def build_program(no_cc=False, debug=False):
    nc = bacc.Bacc("TRN2", num_devices=NCORES)

    # ---- I/O -------------------------------------------------------------
    x_l = nc.dram_tensor("x_l", [BL, F], F32, kind="ExternalInput")
    nb_l = nc.dram_tensor("nb_l", [BL * N, F], F32, kind="ExternalInput")
    attp = nc.dram_tensor("attp", [2, F], F32, kind="ExternalInput")
    wpk = nc.dram_tensor("wpk", [H, WPKC], F32, kind="ExternalInput")
    wbig = nc.dram_tensor("wbig", [P, H * F // P * NCLS + P + NJ * 32], F16,
                          kind="ExternalInput")
    bcc = nc.dram_tensor("bcc", [NCLS, 1], F32, kind="ExternalInput")
    cst = nc.dram_tensor("cst", [32, NJ + P], F32, kind="ExternalInput")
    out_l = nc.dram_tensor("out_l", [BL, NCLS], F32, kind="ExternalOutput")
    if debug:
        dbg_w = nc.dram_tensor("dbg_w", [32, F], F16, kind="ExternalOutput")
        dbg_tT = nc.dram_tensor("dbg_tT", [P, P], F32, kind="ExternalOutput")
        dbg_dT = nc.dram_tensor("dbg_dT", [P, P], F32, kind="ExternalOutput")
        dbg_z1T = nc.dram_tensor("dbg_z1T", [P, P], F32,
                                 kind="ExternalOutput")
        dbg_as0 = nc.dram_tensor("dbg_as0", [P, FC, F], F16,
                                 kind="ExternalOutput")
        dbg_ab = nc.dram_tensor("dbg_ab", [P, 2 * H], F32,
                                kind="ExternalOutput")
        dbg_z2T = nc.dram_tensor("dbg_z2T", [P, FC, BL, H], F16,
                                 kind="ExternalOutput")
        dbg_ptil = nc.dram_tensor("dbg_ptil", [P, P, H], F16,
                                  kind="ExternalOutput")
        dbg_qs = nc.dram_tensor("dbg_qs", [P, 4, F], F16,
                                kind="ExternalOutput")

    with tile.TileContext(nc) as tc, ExitStack() as ctx:
        sg = ctx.enter_context(tc.tile_pool(name="singles", bufs=1))
        wk = ctx.enter_context(tc.tile_pool(name="work", bufs=2))
        tl = ctx.enter_context(tc.tile_pool(name="tail", bufs=2))
        dr = ctx.enter_context(tc.tile_pool(name="dram", bufs=1, space="DRAM"))
        s1ctx = ExitStack()
        s1 = s1ctx.enter_context(tc.tile_pool(name="stage1", bufs=1))
        s1w = s1ctx.enter_context(tc.tile_pool(name="s1work", bufs=3))
        p1 = s1ctx.enter_context(tc.tile_pool(name="psum1", bufs=1,
                                              space="PSUM"))


        V, S, G = nc.vector, nc.scalar, nc.gpsimd
        TE = nc.tensor

        # ---- startup DMAs (priority order: x, att, neighbors, weights) ---
        xsb = s1.tile([BL, F], F32)
        nc.sync.dma_start(xsb[:], x_l[:])
        attsb2 = s1.tile([1, F], F32)
        nc.sync.dma_start(attsb2[:], attp[1:2, :])
        nb_r = nb_l[:].rearrange("(j p) f -> j p f", p=P)
        nbts = []
        for j in range(4):
            nbt = s1w.tile([P, F], F32, tag="nbt", bufs=4, name=f"nbt{j}")
            nc.sync.dma_start(nbt[:], nb_r[j])
            nbts.append(nbt)
        wpks = sg.tile([H, WPKC], F32)
        nc.sync.dma_start(wpks[:], wpk[:])
        wbigs = sg.tile([P, H * F // P * NCLS + P + NJ * 32], F16)
        nc.sync.dma_start(wbigs[:], wbig[:])
        bccs = sg.tile([NCLS, 1], F32)
        nc.sync.dma_start(bccs[:], bcc[:])
        csts = sg.tile([32, NJ + P], F32)
        nc.sync.dma_start(csts[:], cst[:])
        wct_sb = wbigs[:, 0:H * F // P * NCLS].rearrange(
            "p (jc n) -> p jc n", n=NCLS)
        bd0 = wbigs[:, H * F // P * NCLS:H * F // P * NCLS + P]
        mask8 = wbigs[:, H * F // P * NCLS + P:].rearrange(
            "p (j c) -> p j c", c=32)
        jmask = csts[:, 0:NJ]
        a32 = csts[:, NJ:]

        # ---- constants ---------------------------------------------------
        i32h = sg.tile([32, 32], F16)
        make_identity(nc, i32h[:])
        i128h = sg.tile([P, P], F16)
        make_identity(nc, i128h[:])
        i16f = sg.tile([H, H], F32)
        make_identity(nc, i16f[:])
        i64f = sg.tile([NCLS, NCLS], F32)
        make_identity(nc, i64f[:])
        i16big = sg.tile([H, P], F32)
        for i in range(8):
            V.tensor_copy(i16big[:, H * i:H * i + H], i16f[:])
        ones128 = sg.tile([P, 1], F32)
        V.memset(ones128[:], 1.0)
        ones128h = sg.tile([P, 1], F16)
        V.memset(ones128h[:], 1.0)
        onesrow = sg.tile([1, P], F32)
        V.memset(onesrow[:], 1.0)
        onesrowh = sg.tile([1, P], F16)
        V.memset(onesrowh[:], 1.0)
        epsb = sg.tile([H, 1], F32)
        V.memset(epsb[:], EPS_BN)

        # ---- stage 0: u = sign(x), s_x, broadcast helpers ----------------
        u16 = s1.tile([BL, F], F16)
        S.activation(u16[:], xsb[:], AF.Sign)
        p_att2 = p1.tile([P, F], F32)
        TE.matmul(p_att2[:], onesrow[:], attsb2[:], start=True, stop=True)
        att2_b = s1.tile([P, F], F32)
        S.activation(att2_b[:], p_att2[:], AF.Copy)
        # uT16[p, (c b)] = u[b, 128c+p]
        p_ut = p1.tile([P, FC, 32], F16, tag="ptr")
        for c in range(FC):
            TE.transpose(p_ut[:, c, :], u16[:, P * c:P * c + P], i32h[:])
        uT16 = sg.tile([P, P], F16)
        V.tensor_copy(uT16[:].rearrange("p (c b) -> p c b", c=FC), p_ut[:])
        # s_x[b] = sum_f u[b,f] att1[f] via PE over transposed u
        att1c = s1.tile([P, FC], F32)
        nc.sync.dma_start(att1c[:], _bc_ap(attp[:], [[1, P], [P, FC]]))
        att1c16 = s1.tile([P, FC], F16)
        V.tensor_copy(att1c16[:], att1c[:])
        p_sxc = p1.tile([32, 1], F32, tag="pcol")
        for c in range(FC):
            TE.matmul(p_sxc[:], uT16[:, 32 * c:32 * c + 32],
                      att1c16[:, c:c + 1], start=(c == 0), stop=(c == FC - 1))
        sx_col = s1.tile([BL, 1], F32)
        V.tensor_copy(sx_col[:], p_sxc[:])
        sxm = s1.tile([32, NJ], F32)
        V.tensor_tensor(sxm[:], sx_col[:].to_broadcast([32, NJ]), jmask,
                        ALU.mult)
        p_sx = p1.tile([P, NJ], F32, tag="pbig")
        TE.matmul(p_sx[:], a32, sxm[:], start=True, stop=True)
        sx_rep = s1.tile([P, NJ], F32)
        V.tensor_copy(sx_rep[:], p_sx[:])

        # ---- stage 1: neighbor signs, logits, softmax, v -----------------
        sy8 = s1.tile([P, NJ], F32)
        lcol8 = s1.tile([P, NJ], F32)
        ecol8 = s1.tile([P, NJ], F32)
        p_dn = p1.tile([32, 1], F32, tag="pcol")
        p_v = p1.tile([32, F], F32, tag="pbig")
        for j in range(NJ):
            if j < 4:
                nbt = nbts[j]
            else:
                nbt = s1w.tile([P, F], F32, tag="nbt", bufs=4,
                               name=f"nbt{j}")
                nc.sync.dma_start(nbt[:], nb_r[j])
            sgn = s1w.tile([P, F], F16, tag="sgn", bufs=4)
            S.activation(sgn[:], nbt[:], AF.Sign)
            sydump = s1w.tile([P, F], F16, tag="sydump", bufs=1)
            V.scalar_tensor_tensor(sydump[:], sgn[:], 0.0,
                                   att2_b[:], ALU.bypass, ALU.mult,
                                   accum_out=sy8[:, j:j + 1])
            V.tensor_tensor(lcol8[:, j:j + 1], sy8[:, j:j + 1],
                            sx_rep[:, j:j + 1], ALU.mult)
            S.activation(ecol8[:, j:j + 1], lcol8[:, j:j + 1], AF.Exp)
            wd32 = s1w.tile([P, 32], F16, tag="wd32")
            V.tensor_tensor(wd32[:], ecol8[:, j:j + 1].to_broadcast([P, 32]),
                            mask8[:, j, :], ALU.mult)  # mask8 is f16 view
            TE.matmul(p_dn[:], wd32[:], ones128h[:],
                      start=(j == 0), stop=(j == NJ - 1))
            TE.matmul(p_v[:], wd32[:], sgn[:],
                      start=(j == 0), stop=(j == NJ - 1))
        rdn32 = s1.tile([32, 1], F32)
        V.reciprocal(rdn32[:], p_dn[:])
        # w16 = (v * 1/dn) * u  in one pass
        w16_all = s1.tile([32, F], F16)
        V.scalar_tensor_tensor(w16_all[:], p_v[:], rdn32[:], u16[:],
                               ALU.mult, ALU.mult)
        w16_d = dr.tile([32, F], F16)
        nc.sync.dma_start(w16_d[:], w16_all[:])
        if debug:
            nc.sync.dma_start(dbg_w[:], w16_all[:])
        # wT_all[p, c, b] = w[b, 128c+p]
        p_wt = p1.tile([P, FC, 32], F16, tag="ptr")
        for c in range(FC):
            TE.transpose(p_wt[:, c, :], w16_all[:, P * c:P * c + P], i32h[:])
        wT_all = sg.tile([P, FC, 32], F32)
        V.tensor_copy(wT_all[:], p_wt[:])
        s1ctx.close()

        # ---- stage 2: A matrix, t/d column sums, As cache ----------------
        ps2ctx = ExitStack()
        ps2 = ps2ctx.enter_context(tc.tile_pool(name="psum2", bufs=1,
                                                space="PSUM"))
        as_cache = sg.tile([P, FC, BL, F], F16)
        pt = ps2.tile([P, FC, BL], F32)
        pd = ps2.tile([P, FC, BL], F32)
        p_wbc0 = ps2.tile([P, F], F32)
        for b in range(BL):
            w_bc = wk.tile([P, F], F16, tag="wbc", bufs=3)
            if b == 0:
                TE.matmul(p_wbc0[:], onesrowh[:], w16_all[0:1, :],
                          start=True, stop=True)
                S.activation(w_bc[:], p_wbc0[:], AF.Copy)
            else:
                nc.sync.dma_start(w_bc[:], _bc_ap(w16_d[:], [[0, P], [1, F]],
                                                  extra_off=b * F))
            t4 = wk.tile([P, FC, F], F16, tag="t4")
            for c in range(FC):
                V.tensor_scalar(t4[:, c, :], w_bc[:], wT_all[:, c, b:b + 1],
                                None, ALU.add)
            asb = as_cache[:, :, b, :]
            V.tensor_scalar(asb.bitcast(U16), t4[:].bitcast(U16),
                            0x8000, 0x3C00, ALU.bitwise_and, ALU.bitwise_or)
            # |t4| in place: chunks 0-2 on DVE, chunk 3 on Act
            for c in range(3):
                V.tensor_scalar(t4[:, c, :].bitcast(U16),
                                t4[:, c, :].bitcast(U16),
                                0x7FFF, None, ALU.bitwise_and)
            S.activation(t4[:, 3, :], t4[:, 3, :], AF.Abs)
            r4 = wk.tile([P, FC, F], F16, tag="r4")
            S.activation(r4[:], t4[:], AF.Sqrt)
            for c in range(2):
                V.tensor_tensor(as_cache[:, c, b, :], as_cache[:, c, b, :],
                                r4[:, c, :], ALU.mult)
            for c in range(2, FC):
                G.tensor_tensor(as_cache[:, c, b, :], as_cache[:, c, b, :],
                                r4[:, c, :], ALU.mult)
            for kc in range(FC):
                for c in range(FC):
                    TE.matmul(pt[:, kc, b:b + 1],
                              as_cache[:, c, b, P * kc:P * kc + P],
                              ones128h[:], start=(c == 0), stop=(c == FC - 1))
                    TE.matmul(pd[:, kc, b:b + 1],
                              r4[:, c, P * kc:P * kc + P],
                              ones128h[:], start=(c == 0), stop=(c == FC - 1))

        # ---- BN1 ---------------------------------------------------------
        tT = sg.tile([P, P], F32)
        V.tensor_copy(tT[:].rearrange("p (c b) -> p c b", c=FC), pt[:])
        dT = sg.tile([P, P], F32)
        V.tensor_copy(dT[:].rearrange("p (c b) -> p c b", c=FC), pd[:])
        ps2ctx.close()
        V.tensor_scalar(dT[:], dT[:], EPS_ROW, None, ALU.add)
        recdT = sg.tile([P, P], F32)
        V.reciprocal(recdT[:], dT[:])
        urdT = sg.tile([P, P], F32)
        V.tensor_tensor(urdT[:], uT16[:], recdT[:], ALU.mult)
        z1T = sg.tile([P, P], F32)
        V.tensor_tensor(z1T[:], tT[:], urdT[:], ALU.mult)
        if debug:
            nc.sync.dma_start(dbg_tT[:], tT[:])
            nc.sync.dma_start(dbg_dT[:], dT[:])
            nc.sync.dma_start(dbg_z1T[:], z1T[:])
            nc.sync.dma_start(dbg_as0[:], as_cache[:, :, 0, :])
        z1sq = tl.tile([P, P], F32, tag="z1sq")
        V.tensor_tensor(z1sq[:], z1T[:], z1T[:], ALU.mult)
        rs = tl.tile([P, 2], F32, tag="rs")
        V.reduce_sum(rs[:, 0:1], z1T[:], axis=mybir.AxisListType.X)
        V.reduce_sum(rs[:, 1:2], z1sq[:], axis=mybir.AxisListType.X)
        psm = ctx.enter_context(tc.tile_pool(name="psmall", bufs=1,
                                             space="PSUM"))
        p_s = psm.tile([1, 2], F32, tag="sma")
        TE.matmul(p_s[:], ones128[:], rs[:], start=True, stop=True)
        s_loc = sg.tile([1, 2], F32)
        V.tensor_copy(s_loc[:], p_s[:])
        cc1_in = dr.tile([1, 2], F32)
        cc1_out = dr.tile([1, 2], F32)
        nc.sync.dma_start(cc1_in[:], s_loc[:])
        if no_cc:
            nc.sync.dma_start(cc1_out[:], cc1_in[:])
        else:
            G.collective_compute("AllReduce", ALU.add,
                                 replica_groups=[list(range(NCORES))],
                                 ins=[cc1_in[:].opt()],
                                 outs=[cc1_out[:].opt()])
        s2 = sg.tile([1, 2], F32)
        nc.sync.dma_start(s2[:], cc1_out[:])
        p_sg = psm.tile([H, 2], F32, tag="sma")
        TE.matmul(p_sg[:], onesrow[0:1, 0:H], s2[:], start=True, stop=True)
        sgb = sg.tile([H, 2], F32)
        V.tensor_copy(sgb[:], p_sg[:])

        # per-channel BN1 affine params: alpha = w1 g1 invsd,
        # beta = be1 - alpha * mz   (written as alpha*(-mz) + be1)
        mzneg = sg.tile([H, 1], F32)
        V.tensor_scalar(mzneg[:], sgb[:, 0:1], -1.0 / NK, None, ALU.mult)
        e2m = sg.tile([H, 1], F32)
        V.tensor_scalar(e2m[:], sgb[:, 1:2], 1.0 / NK, None, ALU.mult)
        mz2 = sg.tile([H, 1], F32)
        V.tensor_tensor(mz2[:], mzneg[:], mzneg[:], ALU.mult)
        varz = sg.tile([H, 1], F32)
        V.tensor_tensor(varz[:], e2m[:], mz2[:], ALU.subtract)
        var1 = sg.tile([H, 1], F32)
        V.tensor_tensor(var1[:], wpks[:, C_W1SQ:C_W1SQ + 1], varz[:],
                        ALU.mult)
        invsd = sg.tile([H, 1], F32)
        S.activation(invsd[:], var1[:], AF.Ln, bias=epsb[:])
        S.activation(invsd[:], invsd[:], AF.Exp, scale=-0.5)
        ab2 = sg.tile([H, 2], F32)
        V.tensor_tensor(ab2[:, 0:1], wpks[:, C_W1G1:C_W1G1 + 1], invsd[:],
                        ALU.mult)
        V.scalar_tensor_tensor(ab2[:, 1:2], ab2[:, 0:1], mzneg[:],
                               wpks[:, C_BE1:C_BE1 + 1], ALU.mult, ALU.add)
        p_ab = psm.tile([1, 2 * H], F32, tag="sma")
        TE.transpose(p_ab[:, 0:H], ab2[:, 0:1], i16f[:])
        TE.transpose(p_ab[:, H:2 * H], ab2[:, 1:2], i16f[:])
        abrow = sg.tile([1, 2 * H], F32)
        V.tensor_copy(abrow[:], p_ab[:])
        p_abb = psm.tile([P, 2 * H], F32, tag="smb")
        TE.matmul(p_abb[:], onesrow[:], abrow[:], start=True, stop=True)
        abb = sg.tile([P, 2 * H], F16)
        V.tensor_copy(abb[:], p_abb[:])
        if debug:
            nc.sync.dma_start(dbg_ab[:], abb[:])
        alpha_b = abb[:, 0:H]
        beta_b = abb[:, H:2 * H]

        # ---- p~ chunks + GT matmuls (interleaved) ------------------------
        pzactx = ExitStack()
        pz = pzactx.enter_context(tc.tile_pool(name="pza", bufs=2,
                                               space="PSUM"))
        midctx = ExitStack()
        mid = midctx.enter_context(tc.tile_pool(name="mid", bufs=1))
        pgtctx = ExitStack()
        pgt = pgtctx.enter_context(tc.tile_pool(name="pgt", bufs=2,
                                                space="PSUM"))
        ptil = mid.tile([P, P, H], F16)
        for c in range(FC):
            sl = slice(32 * c, 32 * c + 32)
            sfull = tl.tile([P, 32, H], F16, tag="sfull")
            V.tensor_tensor(sfull[:],
                            z1T[:, sl, None].to_broadcast([P, 32, H]),
                            alpha_b[:, None, :].to_broadcast([P, 32, H]),
                            ALU.mult)
            V.tensor_tensor(sfull[:], sfull[:],
                            beta_b[:, None, :].to_broadcast([P, 32, H]),
                            ALU.add)
            absS = tl.tile([P, 32, H], F16, tag="absS")
            S.activation(absS[:], sfull[:], AF.Abs)
            S.activation(absS[:], absS[:], AF.Ln, bias=1.0)
            S.activation(absS[:], absS[:], AF.Exp, scale=-1.0)
            V.tensor_tensor(ptil[:, sl, :], sfull[:], absS[:], ALU.mult)
            G.tensor_tensor(ptil[:, sl, :], ptil[:, sl, :],
                            uT16[:, sl, None].to_broadcast([P, 32, H]),
                            ALU.mult)
        # per-g: GT -> z2T scale -> (z2c transpose, W2 matmul, raw q to SBUF)
        # overlapped with the moment accumulations; all cc2-independent
        z2T = sg.tile([P, FC, BL, H], F16)
        qs_all = sg.tile([P, 4, F], F16)
        u4 = urdT[:].rearrange("p (c b) -> p c b", c=FC)
        p_m2 = psm.tile([H, H], F32, tag="sma")
        p_m1 = psm.tile([1, H], F32, tag="smb")
        if debug:
            nc.sync.dma_start(dbg_ptil[:], ptil[:])
        for g in range(4):
            p_gt = pgt.tile([P, FC, 8, H], F32, tag="pgt")
            for bb in range(8):
                b = 8 * g + bb
                for kc in range(FC):
                    for c in range(FC):
                        TE.matmul(p_gt[:, kc, bb, :],
                                  as_cache[:, c, b, P * kc:P * kc + P],
                                  ptil[:, 32 * c + b, :],
                                  start=(c == 0), stop=(c == FC - 1))
            V.tensor_tensor(
                z2T[:, :, 8 * g:8 * g + 8, :], p_gt[:],
                u4[:, :, 8 * g:8 * g + 8, None].to_broadcast([P, FC, 8, H]),
                ALU.mult)
            for kc in range(FC):
                for bb in range(8):
                    b = 8 * g + bb
                    TE.matmul(p_m2[:], z2T[:, kc, b, :], z2T[:, kc, b, :],
                              start=(g == 0 and kc == 0 and bb == 0),
                              stop=(g == 3 and kc == FC - 1 and bb == 7))
                    TE.matmul(p_m1[:], ones128h[:], z2T[:, kc, b, :],
                              start=(g == 0 and kc == 0 and bb == 0),
                              stop=(g == 3 and kc == FC - 1 and bb == 7))
            p_z2c = pz.tile([P, F], F16, tag="pz2c")
            for kc in range(FC):
                TE.transpose(p_z2c[:, P * kc:P * kc + P],
                             z2T[:, kc, 8 * g:8 * g + 8, :], i128h[:])
            z2c = tl.tile([P, F], F16, tag="z2c")
            if g % 2 == 0:
                V.tensor_copy(z2c[:], p_z2c[:])
            else:
                S.activation(z2c[:], p_z2c[:], AF.Copy)
            p_q = pz.tile([P, F], F32, tag="pq")
            TE.matmul(p_q[:], bd0, z2c[:], start=True, stop=True)
            if g % 2 == 0:
                S.activation(qs_all[:, g, :], p_q[:], AF.Copy)
            else:
                V.tensor_copy(qs_all[:, g, :], p_q[:])
        pgtctx.close()
        midctx.close()
        if debug:
            nc.sync.dma_start(dbg_z2T[:], z2T[:])
        pzactx.close()
        m2_sb = sg.tile([H, H], F32)
        V.tensor_copy(m2_sb[:], p_m2[:])
        m1_sb = sg.tile([1, H], F32)
        V.tensor_copy(m1_sb[:], p_m1[:])
        cc2_in = dr.tile([H + 1, H], F32)
        cc2_out = dr.tile([H + 1, H], F32)
        nc.sync.dma_start(cc2_in[0:H, :], m2_sb[:])
        nc.sync.dma_start(cc2_in[H:H + 1, :], m1_sb[:])
        if no_cc:
            nc.sync.dma_start(cc2_out[:], cc2_in[:])
        else:
            G.collective_compute("AllReduce", ALU.add,
                                 replica_groups=[list(range(NCORES))],
                                 ins=[cc2_in[:].opt()],
                                 outs=[cc2_out[:].opt()])
        m2g = sg.tile([H, H], F32)
        nc.sync.dma_start(m2g[:], cc2_out[0:H, :])
        m1col = sg.tile([H, 1], F32)
        nc.sync.dma_start(m1col[:], _bc_ap(cc2_out[:], [[1, H], [1, 1]],
                                           extra_off=H * H))

        # ---- BN2 affine params -------------------------------------------
        w2s = wpks[:, C_W2:C_W2 + H]
        w2ts = wpks[:, C_W2T:C_W2T + H]
        b2col = wpks[:, C_B2:C_B2 + 1]
        p_a1 = psm.tile([H, H], F32, tag="sma")
        TE.matmul(p_a1[:], w2ts, m2g[:], start=True, stop=True)
        a1 = sg.tile([H, H], F32)
        V.tensor_copy(a1[:], p_a1[:])
        t16 = sg.tile([H, H], F32)
        V.tensor_tensor(t16[:], a1[:], w2s, ALU.mult)
        diagq = sg.tile([H, 1], F32)
        V.reduce_sum(diagq[:], t16[:], axis=mybir.AxisListType.X)
        p_wm1 = psm.tile([H, 1], F32, tag="smb")
        TE.matmul(p_wm1[:], w2ts, m1col[:], start=True, stop=True)
        wm1 = sg.tile([H, 1], F32)
        V.tensor_copy(wm1[:], p_wm1[:])
        m2o = sg.tile([H, 1], F32)
        V.tensor_scalar(m2o[:], wm1[:], 1.0 / NK, b2col, ALU.mult, ALU.add)
        tb2 = sg.tile([H, 1], F32)
        V.scalar_tensor_tensor(tb2[:], wm1[:], 2.0 / NK, b2col,
                               ALU.mult, ALU.mult)
        eh2 = sg.tile([H, 1], F32)
        V.tensor_scalar(eh2[:], diagq[:], 1.0 / NK, None, ALU.mult)
        V.tensor_tensor(eh2[:], eh2[:], tb2[:], ALU.add)
        V.tensor_tensor(eh2[:], eh2[:], wpks[:, C_B2SQ:C_B2SQ + 1], ALU.add)
        m2sq = sg.tile([H, 1], F32)
        V.tensor_tensor(m2sq[:], m2o[:], m2o[:], ALU.mult)
        var2 = sg.tile([H, 1], F32)
        V.tensor_tensor(var2[:], eh2[:], m2sq[:], ALU.subtract)
        invsd2 = sg.tile([H, 1], F32)
        S.activation(invsd2[:], var2[:], AF.Ln, bias=epsb[:])
        S.activation(invsd2[:], invsd2[:], AF.Exp, scale=-0.5)
        gd2 = sg.tile([H, 2], F32)
        V.tensor_tensor(gd2[:, 0:1], wpks[:, C_G2:C_G2 + 1], invsd2[:],
                        ALU.mult)
        d0 = sg.tile([H, 1], F32)
        V.tensor_tensor(d0[:], b2col, m2o[:], ALU.subtract)
        V.scalar_tensor_tensor(gd2[:, 1:2], d0[:], gd2[:, 0:1],
                               wpks[:, C_BE2:C_BE2 + 1], ALU.mult, ALU.add)
        p_gd = psm.tile([P, 2], F32, tag="sma")
        TE.matmul(p_gd[:], i16big[:], gd2[:], start=True, stop=True)
        gdrep = sg.tile([P, 2], F32)
        V.tensor_copy(gdrep[:], p_gd[:])

        # ---- q affine (in place) + softsign + classifier -----------------
        for g in range(4):
            V.tensor_scalar(qs_all[:, g, :], qs_all[:, g, :],
                            gdrep[:, 0:1], gdrep[:, 1:2], ALU.mult, ALU.add)
        if debug:
            nc.sync.dma_start(dbg_qs[:], qs_all[:])
        rq = tl.tile([P, 4, F], F16, tag="rq", bufs=1)
        S.activation(rq[:], qs_all[:], AF.Abs)
        S.activation(rq[:], rq[:], AF.Ln, bias=1.0)
        S.activation(rq[:], rq[:], AF.Exp, scale=-1.0)
        V.tensor_tensor(qs_all[:], qs_all[:], rq[:], ALU.mult)
        q8_all = qs_all
        pzb = ctx.enter_context(tc.tile_pool(name="pzb", bufs=1,
                                             space="PSUM"))
        p_oT = psm.tile([NCLS, 4, 8], F32, tag="smb")
        pending = None
        for i in range(H + 1):
            if i < H:
                g, kc = divmod(i, FC)
                p_qt = pzb.tile([P, P], F16, tag="pqt", bufs=4)
                TE.transpose(p_qt[:], q8_all[:, g, P * kc:P * kc + P],
                             i128h[:])
                qt = tl.tile([P, P], F16, tag="qt", bufs=4)
                if i % 2 == 0:
                    V.tensor_copy(qt[:], p_qt[:])
                else:
                    S.activation(qt[:], p_qt[:], AF.Copy)
            if pending is not None:
                pg, pkc, pqt_t = pending
                for o in range(H):
                    TE.matmul(p_oT[:, pg, :], wct_sb[:, o * FC + pkc, :],
                              pqt_t[:, o:P:H],
                              start=(pkc == 0 and o == 0),
                              stop=(pkc == FC - 1 and o == H - 1))
            if i < H:
                pending = (g, kc, qt)
        outT = sg.tile([NCLS, 4, 8], F32)
        V.tensor_scalar(outT[:], p_oT[:], bccs[:], None, ALU.add)
        p_out = psm.tile([BL, NCLS], F32, tag="sma")
        TE.transpose(p_out[:], outT[:].rearrange("n g e -> n (g e)"), i64f[:])
        out_f = sg.tile([BL, NCLS], F32)
        V.tensor_copy(out_f[:], p_out[:])
        nc.sync.dma_start(out_l[:], out_f[:])

    nc.finalize()
    return nc


def kernel(**inputs):
    x = np.asarray(inputs["x"], np.float32)            # [256,1,512]
    nb = np.asarray(inputs["neighbor"], np.float32)    # [256,32,1,512]
    if "prog" not in _CACHE:
        _CACHE["prog"] = build_program()
    nc = _CACHE["prog"]

    w1 = np.asarray(inputs["W1"], np.float32).reshape(H)
    b1 = np.asarray(inputs["b1"], np.float32)
    g1 = np.asarray(inputs["g1"], np.float32)
    be1 = np.asarray(inputs["be1"], np.float32)
    w2 = np.asarray(inputs["W2"], np.float32)
    b2 = np.asarray(inputs["b2"], np.float32)
    g2 = np.asarray(inputs["g2"], np.float32)
    be2 = np.asarray(inputs["be2"], np.float32)
    wc = np.asarray(inputs["Wc"], np.float32)
    bc = np.asarray(inputs["bc"], np.float32)

    wpk = np.zeros((H, WPKC), np.float32)
    wpk[:, C_W1] = w1
    wpk[:, C_B1] = b1
    wpk[:, C_G1] = g1
    wpk[:, C_BE1] = be1
    wpk[:, C_W2:C_W2 + H] = w2
    wpk[:, C_W2T:C_W2T + H] = w2.T
    wpk[:, C_B2] = b2
    wpk[:, C_G2] = g2
    wpk[:, C_BE2] = be2
    wpk[:, C_W1SQ] = w1 * w1
    wpk[:, C_W1G1] = w1 * g1
    wpk[:, C_B2SQ] = b2 * b2

    # wct packed [p, jc*NCLS + n] = WcT[jc*128 + p, n]; bd0 = blockdiag(W2T)
    wct = wc.T.astype(np.float16)                      # [H*F, NCLS]
    wctp = wct.reshape(H * F // P, P, NCLS).transpose(1, 0, 2).reshape(
        P, H * F // P * NCLS)
    bd0 = np.zeros((P, P), np.float16)
    w2t16 = w2.T.astype(np.float16)
    for i in range(8):
        bd0[16 * i:16 * i + 16, 16 * i:16 * i + 16] = w2t16
    # mask8[p, j, c] = 1 iff c == 4j + p//32  (softmax block selector)
    pidx = np.arange(P)[:, None, None]
    jidx = np.arange(NJ)[None, :, None]
    cidx = np.arange(32)[None, None, :]
    mask8 = (cidx == 4 * jidx + pidx // 32).astype(np.float16).reshape(
        P, NJ * 32)
    wbig = np.ascontiguousarray(np.concatenate([wctp, bd0, mask8], axis=1))
    # cst: [32, NJ + 128]: jmask[b, j] = (b//4 == j); a32[b, p] = (b%4 == p//32)
    bidx = np.arange(32)[:, None]
    jmask = (bidx // 4 == np.arange(NJ)[None, :]).astype(np.float32)
    a32 = (bidx % 4 == np.arange(P)[None, :] // 32).astype(np.float32)
    cstm = np.ascontiguousarray(np.concatenate([jmask, a32], axis=1))

    shared = {
        "attp": np.ascontiguousarray(np.stack([
            np.asarray(inputs["att1_w"], np.float32),
            np.asarray(inputs["att2_w"], np.float32)])),
        "wpk": wpk,
        "wbig": wbig,
        "bcc": np.ascontiguousarray(bc[:, None]),
        "cst": cstm,
    }
    in_maps = []
    for c in range(NCORES):
        sl = slice(c * BL, (c + 1) * BL)
        m = dict(shared)
        m["x_l"] = np.ascontiguousarray(x[sl, 0, :])
        m["nb_l"] = np.ascontiguousarray(
            nb[sl, :, 0, :].reshape(BL * N, F))
        in_maps.append(m)

    res = run_bass_kernel_spmd(nc, in_maps, core_ids=list(range(NCORES)))
    return np.concatenate([r["out_l"] for r in res.results], axis=0)
